# revision 1
# baseline (speedup 1.0000x reference)
"""Causal self-attention Bass/Tile kernel for Trainium2, 8 NeuronCores SPMD.

Problem: B=4, T=2048, C=1024, H=16 heads, D=64, f32 in/out.
    qkv = x @ w_qkv.T; per-head causal softmax(q k^T / sqrt(D)) @ v;
    out = attn @ w_out.T + b_out.

Sharding (hybrid batch x tensor-parallel): core c handles batch b = c//2 and
head group hg = c%2 (8 of 16 heads). Each core computes a full [T, C] partial
of the output projection restricted to its heads; the host sums the two
partials per batch and adds the bias.

Per-core device algorithm (all matmuls bf16 x bf16 -> f32 PSUM):
  - qT, kT produced in [j, t] layout, v in [t, j] layout (with an appended
    ones column for the softmax denominators), from the merged xw input
    ([x | w_qkv] per contraction chunk). The input pipe is ONE serialized
    360GB/s channel, so pair-0's projection runs cc-OUTER with 8 PSUM
    accumulation groups open across all 8 banks (the HW limit is one open
    group per bank), consuming each 2-chunk band as it lands.
  - attention is QUERY-BLOCK-OUTER: for each 128-query block ib, the score
    STRIP scT[l, lb<=ib, i in ib] is computed k-stationary in <=8-l-block
    pieces (2-bank PSUM tiles, triple-buffered), exp'd on ScalarE (scale=1/8
    folded, no max subtraction), causal-masked on the diagonal block (DVE),
    then PV accumulates po65[i, 0:65] += ex_strip[lb]^T @ v_aug[lb] into a
    single 1-bank accumulator whose 65th column collects the denominators.
    Strip pieces of ib+1 interleave with PV pieces of ib so the in-order PE
    stream rides ~1 piece behind ScalarE; bufs=3 strips give ScalarE a real
    backlog so PE hiccups (threaded QKV fillers) don't bubble it.
  - po65 drains raw to SBUF per ib (DVE); normalization is batched per
    half-head: reciprocal of the gathered denominators (custom-DVE Newton,
    base 0) and one broadcast (stride-0) multiply into attn_n.
  - attn_n[i, (pair-packed j)] is transposed back to attnT[j, i] with PE
    transpose instructions (bf16 PSUM staging), a head PAIR per [128, 128]
    transpose.
  - output projection from attnT with K=128 chunks; PSUM->SBUF copies
    alternate DVE/ScalarE; y DMA'd per 128-row block, tail split across
    engines/queues.

PSUM budget (8 banks): score strips 3x2, po65/filler/transpose rotation 2x1.
QKV work for later head pairs and the previous pair's transposes are
threaded into the attention stream to fill TensorE gaps.
"""

import sys

if "/opt/trn_rl_repo" not in sys.path:
    sys.path.insert(0, "/opt/trn_rl_repo")

import numpy as np
import ml_dtypes

import concourse.bass as bass
import concourse.tile as tile
import concourse.mybir as mybir
from concourse import bacc
from concourse.bass_utils import run_bass_kernel_spmd

BF16 = mybir.dt.bfloat16
F32 = mybir.dt.float32
NPBF16 = ml_dtypes.bfloat16
EXPF = mybir.ActivationFunctionType.Exp

P = 128
C = 1024
CC = C // P      # 8 contraction chunks
NH = 8           # heads per core
D = 64
J = NH * D       # 512 (local q/k/v width)
JC = J // P      # 4 j-chunks


def build_program(T=2048):
    LC = T // P          # l/t 128-blocks
    NS = T // 512        # 512-wide t-supers
    HB = LC // 2         # half-head block count (8)
    SCALE = 0.125        # 1/sqrt(D)
    XW = T + 3 * J       # merged x|w row width per chunk

    nc = bacc.Bacc("TRN2", target_bir_lowering=False, debug=False, num_devices=8)

    xw_d = nc.dram_tensor("xw", [P, CC, XW], BF16, kind="ExternalInput")
    woutT_d = nc.dram_tensor("woutT", [JC, P, C], BF16, kind="ExternalInput")
    mask_d = nc.dram_tensor("trimask", [P, P], BF16, kind="ExternalInput")
    eye_d = nc.dram_tensor("eye", [P, P], BF16, kind="ExternalInput")
    y_d = nc.dram_tensor("y", [LC, P, C], F32, kind="ExternalOutput")

    with tile.TileContext(nc) as tc:
        with (
            tc.tile_pool(name="persist", bufs=1) as persist,
            tc.tile_pool(name="io", bufs=1) as io_pool,
            tc.tile_pool(name="dn", bufs=2) as dn_pool,
            tc.tile_pool(name="expp", bufs=3) as exp_pool,
            tc.tile_pool(name="outp", bufs=3) as out_pool,
            tc.tile_pool(name="ps_sc", bufs=3, space="PSUM") as ps_sc,
            tc.tile_pool(name="ps_po", bufs=2, space="PSUM") as ps_po,
        ):
            woutT = persist.tile([P, JC, C], BF16)
            trimask = persist.tile([P, P], BF16)
            eye = persist.tile([P, P], BF16)
            qkT = persist.tile([P, 2 * JC, T], BF16)
            v_aug = persist.tile([P, LC, NH, D + 1], BF16)
            # normalized attention in [i, j] layout; head pair p packs its two
            # heads into one 128-wide slab so a single PE transpose covers both
            attn_n = persist.tile([P, LC, JC, P], BF16)
            attnT = persist.tile([P, JC, T], BF16)
            xw = io_pool.tile([P, CC, XW], BF16)

            # banded input DMAs: 2-chunk transfers (the pipe is serialized, so
            # bands just bound the trickle granularity)
            nc.gpsimd.dma_start(trimask[:], mask_d[:])
            nc.gpsimd.dma_start(eye[:], eye_d[:])
            nc.gpsimd.memset(v_aug[:, :, :, D], 1.0)
            nc.sync.dma_start(xw[:, 0:2, :], xw_d[:, 0:2, :])
            nc.scalar.dma_start(xw[:, 2:4, :], xw_d[:, 2:4, :])
            nc.sync.dma_start(xw[:, 4:6, :], xw_d[:, 4:6, :])
            nc.scalar.dma_start(xw[:, 6:8, :], xw_d[:, 6:8, :])
            for jc in range(JC):
                nc.gpsimd.dma_start(woutT[:, jc, :], woutT_d[jc])

            def xcols(cc, c0, n):
                return xw[:, cc, c0 : c0 + n]

            def wcols(cc, c0, n):
                return xw[:, cc, T + c0 : T + c0 + n]

            # ---------------- QKV projection pieces ----------------
            # Filler tiles (qk supers, v blocks, transpose staging) share the
            # 1-bank po rotation so their DVE drains never gate the strips.
            def emit_qk_super(jc, ts):
                """q/k chunk jc, one 512-wide t-super."""
                pq = ps_po.tile([P, 512], F32, tag="po", name=f"qk{jc}_{ts}")
                for cc in range(CC):
                    nc.tensor.matmul(
                        pq[:],
                        wcols(cc, jc * P, P),
                        xcols(cc, ts * 512, 512),
                        start=(cc == 0),
                        stop=(cc == CC - 1),
                    )
                nc.vector.tensor_copy(
                    qkT[:, jc, ts * 512 : (ts + 1) * 512], pq[:]
                )

            def emit_v_block(lc):
                """v for one 128-token block into v_aug[:, lc]."""
                pq = ps_po.tile([P, 512], F32, tag="po", name=f"v{lc}")
                for cc in range(CC):
                    nc.tensor.matmul(
                        pq[:],
                        xcols(cc, lc * P, P),
                        wcols(cc, 2 * J, J),
                        start=(cc == 0),
                        stop=(cc == CC - 1),
                    )
                nc.vector.tensor_copy(
                    v_aug[:, lc, :, 0:D],
                    pq[:].rearrange("p (h d) -> p h d", d=D),
                )

            def emit_transposes(pair, g):
                """attn_n[i, pair] -> attnT[j, i] for 8 i-blocks of one pair."""
                tp = ps_po.tile([P, HB, P], BF16, tag="po", name=f"tp{pair}_{g}")
                for i in range(HB):
                    ib = g * HB + i
                    nc.tensor.transpose(
                        tp[:, i, :], attn_n[:, ib, pair, :], eye[:]
                    )
                nc.vector.tensor_copy(
                    attnT[:, pair, g * HB * P : (g + 1) * HB * P],
                    tp[:].rearrange("p a b -> p (a b)"),
                )

            # pair-0 prologue: cc-OUTER with 8 groups open across all 8 banks
            # (3 sc slots + 2 po slots), consuming each input band as it lands
            t_q01 = ps_sc.tile([P, 2, 512], F32, tag="sc", name="p0_q01")
            t_q23 = ps_sc.tile([P, 2, 512], F32, tag="sc", name="p0_q23")
            t_k01 = ps_sc.tile([P, 2, 512], F32, tag="sc", name="p0_k01")
            t_k2 = ps_po.tile([P, 512], F32, tag="po", name="p0_k2")
            t_k3 = ps_po.tile([P, 512], F32, tag="po", name="p0_k3")
            plan0 = [
                (t_q01[:, 0, :], 0, 0), (t_q01[:, 1, :], 0, 1),
                (t_q23[:, 0, :], 0, 2), (t_q23[:, 1, :], 0, 3),
                (t_k01[:, 0, :], JC, 0), (t_k01[:, 1, :], JC, 1),
                (t_k2[:], JC, 2), (t_k3[:], JC, 3),
            ]
            for cc in range(CC):
                for out_ap, jc, ts in plan0:
                    nc.tensor.matmul(
                        out_ap, wcols(cc, jc * P, P), xcols(cc, ts * 512, 512),
                        start=(cc == 0), stop=(cc == CC - 1),
                    )
            # copies ordered so strip (ib=0) — q block 0 + k block 0 — goes first
            nc.vector.tensor_copy(
                qkT[:, 0, 0:1024], t_q01[:].rearrange("p a b -> p (a b)"))
            nc.vector.tensor_copy(
                qkT[:, JC, 0:1024], t_k01[:].rearrange("p a b -> p (a b)"))
            nc.vector.tensor_copy(
                qkT[:, 0, 1024:2048], t_q23[:].rearrange("p a b -> p (a b)"))
            nc.vector.tensor_copy(qkT[:, JC, 1024:1536], t_k2[:])
            nc.vector.tensor_copy(qkT[:, JC, 1536:2048], t_k3[:])
            emit_v_block(0)

            # insertion plan: an item at position p is emitted after query
            # block p's PV. Head 0 carries the v projection (v block lc must
            # land before PV first touches l-block lc, i.e. before pv(ib=lc));
            # later heads carry the next pair's qk supers (q in the earlier
            # head, k in the later); head 2p+2 carries pair p's transposes;
            # pair 3's first transpose half fits inside head 7.
            inserts = {hh: [] for hh in range(NH)}
            for lc in range(1, LC):
                inserts[0].append((lc - 1, ("v", lc)))
            for pair in range(1, JC):
                qh = 1 if pair == 1 else 2 * pair - 2
                kh = 1 if pair == 1 else 2 * pair - 1
                for ts in range(NS):
                    inserts[qh].append((2 + 3 * ts if pair == 1 else 3 + 3 * ts,
                                        ("qk", (pair, ts))))
                    inserts[kh].append((11 + ts if pair == 1 else 2 + 3 * ts,
                                        ("qk", (JC + pair, ts))))
            for pair in range(JC - 1):
                inserts[2 * pair + 2].append((5, ("tp", (pair, 0))))
                inserts[2 * pair + 2].append((8, ("tp", (pair, 1))))
            inserts[7].append((10, ("tp", (3, 0))))
            for hh in range(NH):
                inserts[hh].sort(key=lambda it: it[0])

            # ---------------- attention (query-block outer) ----------------
            def strip_pieces(ib):
                """(offset, n) l-block pieces of query block ib's score strip."""
                n = ib + 1
                return [(0, n)] if n <= HB else [(0, HB), (HB, n - HB)]

            for h in range(NH):
                bp = (h % 2) * 64
                chq = h // 2
                qTh = qkT[bp : bp + 64, chq, :]
                kTh = qkT[bp : bp + 64, JC + chq, :]
                attn_raw = dn_pool.tile([P, LC, D + 1], F32, tag="ar",
                                        name=f"ar{h}")
                dens = dn_pool.tile([P, LC], F32, tag="dn", name=f"dn{h}")
                recips = dn_pool.tile([P, LC], F32, tag="rc", name=f"rc{h}")
                rscr = dn_pool.tile([P, LC], F32, tag="rs", name=f"rs{h}")

                insert_at = {}
                for ib_at, item in inserts[h]:
                    insert_at.setdefault(ib_at, []).append(item)

                def emit_strip_piece(ib, off, n):
                    """scores + exp (+ diag mask) for l-blocks [off, off+n)
                    of query block ib; returns the bf16 exp strip."""
                    st = ps_sc.tile([P, HB, P], F32, tag="sc",
                                    name=f"st{h}_{ib}_{off}")
                    for i in range(n):
                        lb = off + i
                        nc.tensor.matmul(
                            st[:, i, :],
                            kTh[:, lb * P : (lb + 1) * P],
                            qTh[:, ib * P : (ib + 1) * P],
                            start=True, stop=True,
                        )
                    exs = exp_pool.tile([P, HB, P], BF16, tag="ex",
                                        name=f"ex{h}_{ib}_{off}")
                    nc.scalar.activation(
                        exs[:, 0:n, :].rearrange("p a b -> p (a b)"),
                        st[:, 0:n, :].rearrange("p a b -> p (a b)"),
                        EXPF, scale=SCALE,
                    )
                    if off <= ib < off + n:
                        nc.vector.tensor_mul(
                            exs[:, ib - off, :], exs[:, ib - off, :], trimask[:]
                        )
                    return exs

                def emit_pv_piece(ib, po65, exs, off, n):
                    for i in range(n):
                        lb = off + i
                        nc.tensor.matmul(
                            po65[:], exs[:, i, :], v_aug[:, lb, h, :],
                            start=(lb == 0), stop=(lb == ib),
                        )

                def emit_norm(half):
                    """normalize query blocks [half*8, half*8+8) of this head."""
                    s = half * HB
                    e = s + HB
                    nc.vector.tensor_copy(dens[:, s:e], attn_raw[:, s:e, D])
                    nc.vector.reciprocal_approx_accurate(
                        recips[:, s:e], dens[:, s:e], rscr[:, s:e]
                    )
                    nc.vector.tensor_mul(
                        attn_n[:, s:e, chq, bp : bp + 64],
                        attn_raw[:, s:e, 0:D],
                        recips[:, s:e].broadcast_to((P, HB, D)),
                    )

                # piece-granular software pipeline: the strip pieces of query
                # block ib+1 interleave with the PV pieces of block ib.
                prev = [(0, exs_piece, off, n)
                        for (off, n) in strip_pieces(0)
                        for exs_piece in [emit_strip_piece(0, off, n)]]
                po_prev = None
                for ib in range(LC):
                    nxt = []
                    sp_next = strip_pieces(ib + 1) if ib + 1 < LC else []
                    po65 = ps_po.tile([P, D + 1], F32, tag="po",
                                      name=f"po{h}_{ib}")
                    npv = len(prev)
                    for j in range(max(npv, len(sp_next))):
                        if j < len(sp_next):
                            off, n = sp_next[j]
                            nxt.append((ib + 1, emit_strip_piece(ib + 1, off, n),
                                        off, n))
                        if j < npv:
                            _, exs, off, n = prev[j]
                            emit_pv_piece(ib, po65, exs, off, n)
                    nc.vector.tensor_copy(attn_raw[:, ib, :], po65[:])
                    for kind, arg in insert_at.get(ib, []):
                        if kind == "v":
                            emit_v_block(arg)
                        elif kind == "qk":
                            emit_qk_super(*arg)
                        else:
                            emit_transposes(*arg)
                    if ib == HB - 1:
                        emit_norm(0)
                    prev = nxt
                emit_norm(1)

            # last transpose half can't hide in a later head
            emit_transposes(JC - 1, 1)

            # ---------------- output projection ----------------
            for tb in range(LC):
                po_ = ps_sc.tile([P, 2, 512], F32, tag="sc", name=f"o_ps{tb}")
                for oc in range(2):
                    for jc in range(JC):
                        nc.tensor.matmul(
                            po_[:, oc, :],
                            attnT[:, jc, tb * P : (tb + 1) * P],
                            woutT[:, jc, oc * 512 : (oc + 1) * 512],
                            start=(jc == 0),
                            stop=(jc == JC - 1),
                        )
                ot = out_pool.tile([P, C], F32, tag="ot", name=f"ot{tb}")
                po_flat = po_[:].rearrange("p a b -> p (a b)")
                if tb < LC - 2:
                    if tb % 2 == 0:
                        nc.vector.tensor_copy(ot[:], po_flat)
                    else:
                        nc.scalar.copy(ot[:], po_flat)
                    nc.sync.dma_start(y_d[tb], ot[:])
                else:
                    # tail: split the copy across DVE+ACT and the DMA across
                    # two queues so the drain after the last matmul is short
                    nc.vector.tensor_copy(ot[:, 0:512], po_flat[:, 0:512])
                    nc.scalar.copy(ot[:, 512:1024], po_flat[:, 512:1024])
                    nc.sync.dma_start(y_d[tb, :, 0:512], ot[:, 0:512])
                    nc.scalar.dma_start(y_d[tb, :, 512:1024], ot[:, 512:1024])

    nc.compile()
    return nc


_CACHE = {}

# Set by test harnesses to capture a profile; harmless defaults for grading.
TRACE = False
LAST_RESULT = None


def get_program(T=2048):
    if T not in _CACHE:
        _CACHE[T] = build_program(T)
    return _CACHE[T]


def make_in_map(x_b, w_qkv, w_out, hg, T=2048):
    """Host-side shard prep for one core: batch slice x_b [T, C], head group hg."""
    xT = np.ascontiguousarray(x_b.T).astype(NPBF16).reshape(CC, P, T)
    W = np.concatenate(
        [
            w_qkv[hg * J : (hg + 1) * J],
            w_qkv[C + hg * J : C + (hg + 1) * J],
            w_qkv[2 * C + hg * J : 2 * C + (hg + 1) * J],
        ],
        axis=0,
    )  # [3J, C]
    wqkvT = np.ascontiguousarray(W.T).astype(NPBF16).reshape(CC, P, 3 * J)
    # partition-major merged layout so banded multi-chunk DMAs are contiguous
    xw = np.ascontiguousarray(
        np.concatenate([xT, wqkvT], axis=2).transpose(1, 0, 2)
    )  # [P, CC, T + 3J]
    Wo = w_out[:, hg * J : (hg + 1) * J]  # [C, J]
    woutT = np.ascontiguousarray(Wo.T).astype(NPBF16).reshape(JC, P, C)
    tri = np.triu(np.ones((P, P), np.float32)).astype(NPBF16)
    eye = np.eye(P, dtype=np.float32).astype(NPBF16)
    return {"xw": xw, "woutT": woutT, "trimask": tri, "eye": eye}


def kernel(x, w_qkv, w_out, b_out):
    x = np.asarray(x, dtype=np.float32)
    w_qkv = np.asarray(w_qkv, dtype=np.float32)
    w_out = np.asarray(w_out, dtype=np.float32)
    b_out = np.asarray(b_out, dtype=np.float32)
    B, T, Cx = x.shape
    assert Cx == C

    nc = get_program(T)
    in_maps = [
        make_in_map(x[core // 2], w_qkv, w_out, core % 2, T) for core in range(8)
    ]
    res = run_bass_kernel_spmd(nc, in_maps, core_ids=list(range(8)), trace=TRACE)
    global LAST_RESULT
    LAST_RESULT = res
    outs = [r["y"].reshape(T, C).astype(np.float32) for r in res.results]
    y = np.stack([outs[2 * b] + outs[2 * b + 1] for b in range(B)])
    return (y + b_out[None, None, :]).astype(np.float32)



# revision 25
# speedup vs baseline: 1.0816x; 1.0816x over previous
"""Causal self-attention Bass/Tile kernel for Trainium2, 8 NeuronCores SPMD.

Problem: B=4, T=2048, C=1024, H=16 heads, D=64, f32 in/out.
    qkv = x @ w_qkv.T; per-head causal softmax(q k^T / sqrt(D)) @ v;
    out = attn @ w_out.T + b_out.

Sharding (hybrid batch x tensor-parallel): core c handles batch b = c//2 and
head group hg = c%2 (8 of 16 heads). Each core computes a full [T, C] partial
of the output projection restricted to its heads; the host sums the two
partials per batch (bf16 partials, f32 sum) and adds the bias.

Per-core device algorithm (all matmuls bf16 x bf16 -> f32 PSUM):
  - Inputs arrive as separate w (j-chunk-major) and x (t-band) streams so the
    first score strip can issue ~4us in: w[q0], w[k0], x[t<128] land first,
    then v-chunk weights and the remaining x bands trickle in while head 0
    runs.  qkT is produced in [j, t] layout by cc-inner supers per x band;
    v in [t, j] layout per (block, chunk) with an appended ones column for
    the softmax denominators.
  - Attention is a single flat pipeline over items (h, ib) with a LEAD-2
    strip prefetch: at item k the score strip of item k+2 is computed
    (k-stationary, <=8-l-block pieces in 2-bank PSUM tiles), exp'd on
    ScalarE (scale=1/8 folded, no max subtraction), causal-masked on the
    diagonal block (DVE), while the PV of item k accumulates
    po65[i, 0:65] += ex_strip[lb]^T @ v_aug[lb] into a 1-bank accumulator
    whose 65th column collects the denominators.  The 2-item lead gives
    ScalarE a deep backlog so PV never waits on exp.
  - po65 drains raw to SBUF per ib (DVE); normalization is batched per
    half-head: reciprocal of the gathered denominators and one broadcast
    multiply into attn_n.
  - attn_n[i, (pair-packed j)] is transposed back to attnT[j, i] with PE
    transpose instructions (bf16 PSUM staging), a head PAIR per [128, 128]
    transpose; transposes thread into heads 4-7.
  - QKV work for later head pairs, v chunks, and transposes are threaded
    into the item stream as inserts to fill TensorE gaps and respect the
    input-DMA pacing.
  - Output projection from attnT with K=128 chunks; PSUM->SBUF copies
    alternate DVE/ACT; y is stored/DMA'd in BF16 (host sums partials in
    f32), halving the output-DMA tail.

PSUM budget (8 banks): score strips 3x2 (shared with out-proj psum), po65/
filler/transpose rotation 2x1.
"""

import sys

if "/opt/trn_rl_repo" not in sys.path:
    sys.path.insert(0, "/opt/trn_rl_repo")

import numpy as np
import ml_dtypes

import concourse.bass as bass
import concourse.tile as tile
import concourse.mybir as mybir
from concourse import bacc
from concourse.bass_utils import run_bass_kernel_spmd

BF16 = mybir.dt.bfloat16
F32 = mybir.dt.float32
NPBF16 = ml_dtypes.bfloat16
EXPF = mybir.ActivationFunctionType.Exp

P = 128
C = 1024
CC = C // P      # 8 contraction chunks
NH = 8           # heads per core
D = 64
J = NH * D       # 512 (local q/k/v width)
JC = J // P      # 4 j-chunks
WC = 3 * JC      # 12 w j-chunks (q 0-3, k 4-7, v 8-11)

LEAD = 4         # strip prefetch depth (items)


def build_program(T=2048):
    LC = T // P          # l/t 128-blocks
    NS = T // 512        # 512-wide t-supers
    HB = LC // 2         # half-head block count (8)
    SCALE = 0.125        # 1/sqrt(D)

    nc = bacc.Bacc("TRN2", target_bir_lowering=False, debug=False, num_devices=8)

    w_d = nc.dram_tensor("wj", [P, CC, 3 * J], BF16, kind="ExternalInput")
    x_d = nc.dram_tensor("xt", [P, CC, T], BF16, kind="ExternalInput")
    woutT_d = nc.dram_tensor("woutT", [P, JC, C], BF16, kind="ExternalInput")
    mask_d = nc.dram_tensor("trimask", [P, P], BF16, kind="ExternalInput")
    eye_d = nc.dram_tensor("eye", [P, P], BF16, kind="ExternalInput")
    y_d = nc.dram_tensor("y", [LC, P, C], BF16, kind="ExternalOutput")

    # x DMA bands (t ranges) and the q0/k0 super cuts that consume them
    XBANDS = [(0, 128), (128, 512), (512, 1024), (1024, 1536), (1536, 2048)]

    with tile.TileContext(nc) as tc:
        with (
            tc.tile_pool(name="persist", bufs=1) as persist,
            tc.tile_pool(name="dn", bufs=2) as dn_pool,
            tc.tile_pool(name="expp", bufs=10) as exp_pool,
            tc.tile_pool(name="outp", bufs=3) as out_pool,
            tc.tile_pool(name="ps_sc", bufs=3, space="PSUM") as ps_sc,
            tc.tile_pool(name="ps_po", bufs=2, space="PSUM") as ps_po,
        ):
            w_sb = persist.tile([P, CC, 3 * J], BF16)
            x_sb = persist.tile([P, CC, T], BF16)
            woutT = persist.tile([P, JC, C], BF16)
            trimask = persist.tile([P, P], BF16)
            eye = persist.tile([P, P], BF16)
            qkT = persist.tile([P, 2 * JC, T], BF16)
            v_aug = persist.tile([P, LC, NH, D + 1], BF16)
            # normalized attention in [i, j] layout; head pair p packs its two
            # heads into one 128-wide slab so a single PE transpose covers both
            attn_n = persist.tile([P, LC, JC, P], BF16)
            attnT = persist.tile([P, JC, T], BF16)

            ones1 = persist.tile([P, 1], BF16)
            nc.vector.memset(ones1[:], 1.0)
            nc.vector.tensor_copy(
                v_aug[:, :, :, D], ones1[:, 0].broadcast_to((P, LC, NH))
            )

            # ---- input DMA stream, ordered for earliest first strip ----
            # w_sb column order: [q0 k0 | v0..v3 | q1 k1 | q2 k2 | q3 k3] so
            # priority ranges are contiguous.  One queue = strict DMA order;
            # each dma_start costs ~0.6us of descriptor generation, so keep
            # the count low and front-load only what unblocks compute.
            def wslice(c0, c1):
                nc.sync.dma_start(w_sb[:, :, c0:c1], w_d[:, :, c0:c1])

            def xslice(t0, t1):
                nc.sync.dma_start(x_sb[:, :, t0:t1], x_d[:, :, t0:t1])

            nc.scalar.dma_start(trimask[:], mask_d[:])
            wslice(0, 128)            # q0
            xslice(0, 128)
            wslice(128, 256)          # k0
            wslice(256, 384)          # v chunk 0 (heads 0-1)
            xslice(128, 512)
            xslice(512, 1024)
            wslice(768, 1024)         # q1, k1
            xslice(1024, 1536)
            wslice(384, 768)          # v chunks 1-3
            xslice(1536, 2048)
            wslice(1024, 1536)        # q2 k2 q3 k3
            nc.scalar.dma_start(eye[:], eye_d[:])
            nc.scalar.dma_start(woutT[:], woutT_d[:])

            QCOL = [0, 768, 1024, 1280]
            KCOL = [128, 896, 1152, 1408]
            VCOL = 256

            def wcol(cc, col, n):
                return w_sb[:, cc, col : col + n]

            # ---------------- QKV projection pieces ----------------
            def emit_qk_super(jc, t0, n):
                """qk chunk jc (0-3 q, 4-7 k), t range [t0, t0+n)."""
                col = QCOL[jc] if jc < JC else KCOL[jc - JC]
                pq = ps_po.tile([P, 512], F32, tag="po", name=f"qk{jc}_{t0}")
                for cc in range(CC):
                    nc.tensor.matmul(
                        pq[:, 0:n],
                        wcol(cc, col, P),
                        x_sb[:, cc, t0 : t0 + n],
                        start=(cc == 0),
                        stop=(cc == CC - 1),
                    )
                nc.vector.tensor_copy(qkT[:, jc, t0 : t0 + n], pq[:, 0:n])

            def emit_v(lc, ch, nch=1):
                """v chunks [ch, ch+nch) for one 128-token block lc."""
                pq = ps_po.tile([P, 512], F32, tag="po", name=f"v{lc}_{ch}")
                n = nch * P
                for cc in range(CC):
                    nc.tensor.matmul(
                        pq[:, 0:n],
                        x_sb[:, cc, lc * P : (lc + 1) * P],
                        wcol(cc, VCOL + ch * P, n),
                        start=(cc == 0),
                        stop=(cc == CC - 1),
                    )
                nc.vector.tensor_copy(
                    v_aug[:, lc, 2 * ch : 2 * ch + 2 * nch, 0:D],
                    pq[:, 0:n].rearrange("p (h d) -> p h d", d=D),
                )

            def emit_transposes(pair, g):
                """attn_n[i, pair] -> attnT[j, i] for 8 i-blocks of one pair."""
                tp = ps_po.tile([P, HB, P], BF16, tag="po", name=f"tp{pair}_{g}")
                for i in range(HB):
                    ib = g * HB + i
                    nc.tensor.transpose(
                        tp[:, i, :], attn_n[:, ib, pair, :], eye[:]
                    )
                nc.vector.tensor_copy(
                    attnT[:, pair, g * HB * P : (g + 1) * HB * P],
                    tp[:].rearrange("p a b -> p (a b)"),
                )

            # ---------------- output projection block ----------------
            def emit_outproj_block(tb, act_drain=False, tail=False):
                po_ = ps_sc.tile([P, 2, 512], F32, tag="sc", name=f"o_ps{tb}")
                for oc in range(2):
                    for jc in range(JC):
                        nc.tensor.matmul(
                            po_[:, oc, :],
                            attnT[:, jc, tb * P : (tb + 1) * P],
                            woutT[:, jc, oc * 512 : (oc + 1) * 512],
                            start=(jc == 0),
                            stop=(jc == JC - 1),
                        )
                ot = out_pool.tile([P, C], BF16, tag="ot", name=f"ot{tb}")
                # drain in two 512 halves: the psum slot is held until the
                # copy's deferred ack, so one 1024-wide copy would serialize
                # the 3-slot rotation below the PE pace.  act_drain=False
                # keeps ACT free for exp work (threaded blocks).
                nc.vector.tensor_copy(ot[:, 0:512], po_[:, 0, :])
                if act_drain:
                    nc.scalar.copy(ot[:, 512:1024], po_[:, 1, :])
                else:
                    nc.vector.tensor_copy(ot[:, 512:1024], po_[:, 1, :])
                if not tail:
                    nc.sync.dma_start(y_d[tb], ot[:])
                else:
                    # tail: also split the DMA across two queues so the drain
                    # after the last matmul is short
                    nc.sync.dma_start(y_d[tb, :, 0:512], ot[:, 0:512])
                    nc.scalar.dma_start(y_d[tb, :, 512:1024], ot[:, 512:1024])

            # ---------------- insert plan (global item index) ----------------
            # an entry at key k is emitted after item k's PV drain
            inserts = {}
            NITEM = NH * LC

            def add_ins(idx, fn):
                inserts.setdefault(idx, []).append(fn)

            items = [(h, ib) for h in range(NH) for ib in range(LC)]

            # per-item base PE load (cycles): strip of item idx+LEAD + PV
            load = [0.0] * NITEM
            for idx in range(NITEM):
                ib = items[idx][1]
                load[idx] = (ib + 1) * 65 + 120  # PV + drain slack
                if idx + LEAD < NITEM:
                    load[idx] += (items[idx + LEAD][1] + 1) * 128  # strip

            # forced inserts: v chunk 0 block lc before PV(0, lc); placed a
            # few items early so it isn't stuck behind later-x strip supers
            for lc in range(1, LC):
                add_ins(max(lc - 3, 0), (lambda lc=lc: emit_v(lc, 0)))
                load[max(lc - 3, 0)] += CC * P + 120
            # q0/k0 supers are emitted on demand (ensure_bands); account the
            # load at the item whose strip emission pulls them in
            for ib in range(LC):
                load[max(ib - LEAD, 0)] += 2 * CC * P

            # pinned late-stage work: tp(3,0) right after norm(7,0), then the
            # first 6 out-proj token blocks thread into head 7's tail
            add_ins(121, (lambda: emit_transposes(JC - 1, 0)))
            load[121] += HB * P + 120
            for tb in range(6):
                add_ins(122 + tb, (lambda tb=tb: emit_outproj_block(tb)))
                load[122 + tb] += 2 * JC * 512

            # deadline-balanced inserts: (earliest, latest, cycles, fn)
            # earliest indices approximate when the needed x/w DMA has landed
            xband_idx = {0: 0, 1: 3, 2: 5, 3: 8, 4: 11}

            def lc_band(lc):
                for bi, (t0, t1) in enumerate(XBANDS):
                    if lc * P < t1:
                        return bi
                return len(XBANDS) - 1

            cand = []
            for ch in (1, 2, 3):
                for lc in range(LC):
                    # v(lc, ch) used by PV(2ch, lc) at item 16*2ch + lc
                    earliest = max(xband_idx[lc_band(lc)], 9)
                    cand.append((earliest, 32 * ch + lc - 1, CC * P + 120,
                                 (lambda lc=lc, ch=ch: emit_v(lc, ch))))
            for pair in range(1, JC):
                w_idx = 6 if pair == 1 else 13
                for tsb in range(NS):
                    earliest = max(xband_idx[tsb + 1], w_idx)
                    latest = 32 * pair + 4 * tsb - LEAD - 1
                    cand.append((earliest, latest, CC * 512,
                                 (lambda p=pair, t=tsb:
                                  emit_qk_super(JC + p, t * 512, 512))))
                    cand.append((earliest, latest, CC * 512,
                                 (lambda p=pair, t=tsb:
                                  emit_qk_super(p, t * 512, 512))))
            for pair in range(JC):
                e0 = 16 * (2 * pair + 1) + 8 + 1   # after norm(2p+1, 0)
                e1 = 16 * (2 * pair + 2)           # after norm(2p+1, 1)
                if pair < JC - 1:
                    cand.append((e0, 118, HB * P + 120,
                                 (lambda p=pair: emit_transposes(p, 0))))
                    cand.append((e1, NITEM - 1, HB * P + 120,
                                 (lambda p=pair: emit_transposes(p, 1))))

            cand.sort(key=lambda e: e[1])
            for earliest, latest, cost, fn in cand:
                lo = max(0, min(earliest, NITEM - 1))
                hi = max(lo, min(latest, NITEM - 1))
                k = min(range(lo, hi + 1), key=lambda i: load[i])
                add_ins(k, fn)
                load[k] += cost

            # ---------------- attention (flat item pipeline) ----------------
            def strip_pieces(ib):
                n = ib + 1
                return [(0, n)] if n <= HB else [(0, HB), (HB, n - HB)]

            def qk_head(h):
                bp = (h % 2) * 64
                chq = h // 2
                return (qkT[bp : bp + 64, chq, :], qkT[bp : bp + 64, JC + chq, :])

            def emit_strip_piece(h, ib, off, n):
                """scores + exp (+ diag mask) for l-blocks [off, off+n) of
                query block ib of head h; returns the bf16 exp strip."""
                qTh, kTh = qk_head(h)
                st = ps_sc.tile([P, HB, P], F32, tag="sc",
                                name=f"st{h}_{ib}_{off}")
                for i in range(n):
                    lb = off + i
                    nc.tensor.matmul(
                        st[:, i, :],
                        kTh[:, lb * P : (lb + 1) * P],
                        qTh[:, ib * P : (ib + 1) * P],
                        start=True, stop=True,
                    )
                exs = exp_pool.tile([P, HB, P], BF16, tag="ex",
                                    name=f"ex{h}_{ib}_{off}")
                nc.scalar.activation(
                    exs[:, 0:n, :].rearrange("p a b -> p (a b)"),
                    st[:, 0:n, :].rearrange("p a b -> p (a b)"),
                    EXPF, scale=SCALE,
                )
                if off <= ib < off + n:
                    nc.vector.tensor_mul(
                        exs[:, ib - off, :], exs[:, ib - off, :], trimask[:]
                    )
                return exs

            def emit_pv_piece(h, ib, po65, exs, off, n):
                for i in range(n):
                    lb = off + i
                    nc.tensor.matmul(
                        po65[:], exs[:, i, :], v_aug[:, lb, h, :],
                        start=(lb == 0), stop=(lb == ib),
                    )

            heads = {}

            def head_tiles(h):
                if h not in heads:
                    attn_raw = dn_pool.tile([P, LC, D + 1], F32, tag="ar",
                                            name=f"ar{h}")
                    dens = dn_pool.tile([P, LC], F32, tag="dn", name=f"dn{h}")
                    recips = dn_pool.tile([P, LC], F32, tag="rc", name=f"rc{h}")
                    rscr = dn_pool.tile([P, LC], F32, tag="rs", name=f"rs{h}")
                    heads[h] = (attn_raw, dens, recips, rscr)
                return heads[h]

            def emit_norm(h, half):
                attn_raw, dens, recips, rscr = head_tiles(h)
                bp = (h % 2) * 64
                chq = h // 2
                s = half * HB
                e = s + HB
                nc.vector.tensor_copy(dens[:, s:e], attn_raw[:, s:e, D])
                nc.vector.reciprocal_approx_accurate(
                    recips[:, s:e], dens[:, s:e], rscr[:, s:e]
                )
                nc.vector.tensor_mul(
                    attn_n[:, s:e, chq, bp : bp + 64],
                    attn_raw[:, s:e, 0:D],
                    recips[:, s:e].broadcast_to((P, HB, D)),
                )

            # q0/k0 supers emitted on demand, one 128-col block at a time,
            # right before the strip needing them: fine granularity avoids
            # head-of-line blocking of the in-order PE queue on the x DMA
            qk0_next = [0]

            def ensure_bands(ib):
                while qk0_next[0] <= ib:
                    tb = qk0_next[0]
                    emit_qk_super(0, tb * P, P)
                    emit_qk_super(JC, tb * P, P)
                    qk0_next[0] += 1

            # prologue: first supers + warmup strips for items 0..LEAD-1
            ensure_bands(0)
            emit_v(0, 0)
            from collections import deque

            strip_q = deque()
            for k in range(LEAD):
                h, ib = items[k]
                if h < 2:
                    ensure_bands(ib)
                strip_q.append(
                    [(emit_strip_piece(h, ib, off, n), off, n)
                     for (off, n) in strip_pieces(ib)]
                )

            for idx, (h, ib) in enumerate(items):
                pieces = strip_q.popleft()
                sp_next = []
                if idx + LEAD < len(items):
                    nh, nib = items[idx + LEAD]
                    if nh < 2:
                        ensure_bands(nib)
                    sp_next = [(nh, nib, off, n) for (off, n) in strip_pieces(nib)]
                po65 = ps_po.tile([P, D + 1], F32, tag="po", name=f"po{h}_{ib}")
                nxt = []
                for j in range(max(len(pieces), len(sp_next))):
                    if j < len(sp_next):
                        nh, nib, off, n = sp_next[j]
                        nxt.append((emit_strip_piece(nh, nib, off, n), off, n))
                    if j < len(pieces):
                        exs, off, n = pieces[j]
                        emit_pv_piece(h, ib, po65, exs, off, n)
                if nxt:
                    strip_q.append(nxt)
                attn_raw = head_tiles(h)[0]
                nc.vector.tensor_copy(attn_raw[:, ib, :], po65[:])
                for fn in inserts.get(idx, []):
                    fn()
                if ib == HB - 1:
                    emit_norm(h, 0)
                elif ib == LC - 1:
                    emit_norm(h, 1)

            # last transpose half can't hide in a later head
            emit_transposes(JC - 1, 1)

            # ---------------- output projection (blocks 6+) ----------------
            for tb in range(6, LC):
                emit_outproj_block(tb, act_drain=True, tail=(tb >= LC - 2))

    nc.compile()
    return nc


_CACHE = {}

# Set by test harnesses to capture a profile; harmless defaults for grading.
TRACE = False
LAST_RESULT = None


def get_program(T=2048):
    if T not in _CACHE:
        _CACHE[T] = build_program(T)
    return _CACHE[T]


def make_in_map(x_b, w_qkv, w_out, hg, T=2048):
    """Host-side shard prep for one core: batch slice x_b [T, C], head group hg."""
    xT = np.ascontiguousarray(x_b.T).astype(NPBF16).reshape(CC, P, T)
    x_t = np.ascontiguousarray(xT.transpose(1, 0, 2))  # [P, CC, T]
    wq = w_qkv[hg * J : (hg + 1) * J]                  # [512, C]
    wk = w_qkv[C + hg * J : C + (hg + 1) * J]
    wv = w_qkv[2 * C + hg * J : 2 * C + (hg + 1) * J]
    # w_sb column order: [q0 k0 | v0 v1 v2 v3 | q1 k1 | q2 k2 | q3 k3]
    W = np.concatenate(
        [wq[0:128], wk[0:128], wv]
        + [np.concatenate([wq[p * 128 : (p + 1) * 128],
                           wk[p * 128 : (p + 1) * 128]])
           for p in range(1, JC)],
        axis=0,
    )  # [3J, C]
    # w_d[p, cc, col] = W[col, cc*128+p]
    wj = np.ascontiguousarray(
        W.T.astype(NPBF16).reshape(CC, P, 3 * J).transpose(1, 0, 2)
    )
    Wo = w_out[:, hg * J : (hg + 1) * J]  # [C, J]
    woutT = np.ascontiguousarray(
        Wo.T.astype(NPBF16).reshape(JC, P, C).transpose(1, 0, 2)
    )
    tri = np.triu(np.ones((P, P), np.float32)).astype(NPBF16)
    eye = np.eye(P, dtype=np.float32).astype(NPBF16)
    return {"wj": wj, "xt": x_t, "woutT": woutT, "trimask": tri, "eye": eye}


def kernel(x, w_qkv, w_out, b_out):
    x = np.asarray(x, dtype=np.float32)
    w_qkv = np.asarray(w_qkv, dtype=np.float32)
    w_out = np.asarray(w_out, dtype=np.float32)
    b_out = np.asarray(b_out, dtype=np.float32)
    B, T, Cx = x.shape
    assert Cx == C

    nc = get_program(T)
    in_maps = [
        make_in_map(x[core // 2], w_qkv, w_out, core % 2, T) for core in range(8)
    ]
    res = run_bass_kernel_spmd(nc, in_maps, core_ids=list(range(8)), trace=TRACE)
    global LAST_RESULT
    LAST_RESULT = res
    outs = [r["y"].astype(np.float32).reshape(T, C) for r in res.results]
    y = np.stack([outs[2 * b] + outs[2 * b + 1] for b in range(B)])
    return (y + b_out[None, None, :]).astype(np.float32)


# revision 44
# speedup vs baseline: 1.1417x; 1.0556x over previous
"""Causal self-attention Bass/Tile kernel for Trainium2, 8 NeuronCores SPMD.

Problem: B=4, T=2048, C=1024, H=16 heads, D=64, f32 in/out.
    qkv = x @ w_qkv.T; per-head causal softmax(q k^T / sqrt(D)) @ v;
    out = attn @ w_out.T + b_out.

Sharding (hybrid batch x tensor-parallel): core c handles batch b = c//2 and
head group hg = c%2 (8 of 16 heads). Each core computes a full [T, C] partial
of the output projection restricted to its heads; the host sums the two
partials per batch (bf16 partials, f32 sum) and adds the bias.

Per-core device algorithm (all matmuls bf16 x bf16 -> f32 PSUM):
  - Inputs arrive as separate w (j-chunk-major) and x (t-band) streams so the
    first score strip can issue ~4us in: w[q0], w[k0], x[t<128] land first,
    then v-chunk weights and the remaining x bands trickle in while head 0
    runs.  qkT is produced in [j, t] layout by cc-inner supers per x band;
    v in [t, j] layout per (block, chunk) with an appended ones column for
    the softmax denominators.
  - Attention is a single flat pipeline over items (h, ib) with a LEAD-2
    strip prefetch: at item k the score strip of item k+2 is computed
    (k-stationary, <=8-l-block pieces in 2-bank PSUM tiles), exp'd on
    ScalarE (scale=1/8 folded, no max subtraction), causal-masked on the
    diagonal block (DVE), while the PV of item k accumulates
    po65[i, 0:65] += ex_strip[lb]^T @ v_aug[lb] into a 1-bank accumulator
    whose 65th column collects the denominators.  The 2-item lead gives
    ScalarE a deep backlog so PV never waits on exp.
  - po65 drains raw to SBUF per ib (DVE); normalization is batched per
    half-head: reciprocal of the gathered denominators and one broadcast
    multiply into attn_n.
  - attn_n[i, (pair-packed j)] is transposed back to attnT[j, i] with PE
    transpose instructions (bf16 PSUM staging), a head PAIR per [128, 128]
    transpose; transposes thread into heads 4-7.
  - QKV work for later head pairs, v chunks, and transposes are threaded
    into the item stream as inserts to fill TensorE gaps and respect the
    input-DMA pacing.
  - Output projection from attnT with K=128 chunks; PSUM->SBUF copies
    alternate DVE/ACT; y is stored/DMA'd in BF16 (host sums partials in
    f32), halving the output-DMA tail.

PSUM budget (8 banks): score strips 3x2 (shared with out-proj psum), po65/
filler/transpose rotation 2x1.
"""

import sys

if "/opt/trn_rl_repo" not in sys.path:
    sys.path.insert(0, "/opt/trn_rl_repo")

import numpy as np
import ml_dtypes

import concourse.bass as bass
import concourse.tile as tile
import concourse.mybir as mybir
from concourse import bacc
from concourse.bass_utils import run_bass_kernel_spmd

BF16 = mybir.dt.bfloat16
F32 = mybir.dt.float32
NPBF16 = ml_dtypes.bfloat16
EXPF = mybir.ActivationFunctionType.Exp

P = 128
C = 1024
CC = C // P      # 8 contraction chunks
NH = 8           # heads per core
D = 64
J = NH * D       # 512 (local q/k/v width)
JC = J // P      # 4 j-chunks
WC = 3 * JC      # 12 w j-chunks (q 0-3, k 4-7, v 8-11)

LEAD = 10        # strip prefetch depth (items)


def build_program(T=2048):
    LC = T // P          # l/t 128-blocks
    NS = T // 512        # 512-wide t-supers
    HB = LC // 2         # half-head block count (8)
    SCALE = 0.125        # 1/sqrt(D)

    nc = bacc.Bacc("TRN2", target_bir_lowering=False, debug=False, num_devices=8)

    w_d = nc.dram_tensor("wj", [P, CC, 3 * J], BF16, kind="ExternalInput")
    x_d = nc.dram_tensor("xt", [P, CC, T], BF16, kind="ExternalInput")
    woutT_d = nc.dram_tensor("woutT", [P, JC, C], BF16, kind="ExternalInput")
    mask_d = nc.dram_tensor("trimask", [P, P], BF16, kind="ExternalInput")
    eye_d = nc.dram_tensor("eye", [P, P], BF16, kind="ExternalInput")
    y_d = nc.dram_tensor("y", [LC, P, C], BF16, kind="ExternalOutput")

    # x DMA bands (t ranges); finer bands unlock strip/v work sooner
    XBANDS = [(0, 128), (128, 256), (256, 512), (512, 768), (768, 1024),
              (1024, 1280), (1280, 1536), (1536, 1792), (1792, 2048)]

    with tile.TileContext(nc) as tc:
        with (
            tc.tile_pool(name="persist", bufs=1) as persist,
            tc.tile_pool(name="dn", bufs=2) as dn_pool,
            tc.tile_pool(name="expp", bufs=22) as exp_pool,
            tc.tile_pool(name="outp", bufs=3) as out_pool,
            tc.tile_pool(name="ps_sc", bufs=3, space="PSUM") as ps_sc,
            tc.tile_pool(name="ps_po", bufs=2, space="PSUM") as ps_po,
        ):
            w_sb = persist.tile([P, CC, 3 * J], BF16)
            x_sb = persist.tile([P, CC, T], BF16)
            woutT = persist.tile([P, JC, C], BF16)
            trimask = persist.tile([P, P], BF16)
            eye = persist.tile([P, P], BF16)
            qkT = persist.tile([P, 2 * JC, T], BF16)
            v_aug = persist.tile([P, LC, NH, D + 1], BF16)
            # normalized attention in [i, j] layout; head pair p packs its two
            # heads into one 128-wide slab so a single PE transpose covers both
            attn_n = persist.tile([P, LC, JC, P], BF16)
            attnT = persist.tile([P, JC, T], BF16)

            ones1 = persist.tile([P, 1], BF16)
            nc.vector.memset(ones1[:], 1.0)
            nc.vector.tensor_copy(
                v_aug[:, :, :, D], ones1[:, 0].broadcast_to((P, LC, NH))
            )

            # ---- input DMA stream, ordered for earliest first strip ----
            # w_sb column order: [q0 k0 | v0..v3 | q1 k1 | q2 k2 | q3 k3] so
            # priority ranges are contiguous.  One queue = strict DMA order;
            # each dma_start costs ~0.6us of descriptor generation, so keep
            # the count low and front-load only what unblocks compute.
            def wslice(c0, c1):
                nc.sync.dma_start(w_sb[:, :, c0:c1], w_d[:, :, c0:c1])

            def xslice(t0, t1):
                nc.sync.dma_start(x_sb[:, :, t0:t1], x_d[:, :, t0:t1])

            nc.scalar.dma_start(trimask[:], mask_d[:])
            wslice(0, 128)            # q0
            xslice(0, 128)
            wslice(128, 256)          # k0
            wslice(256, 384)          # v chunk 0 (heads 0-1)
            xslice(128, 256)
            xslice(256, 512)
            xslice(512, 768)
            xslice(768, 1024)
            wslice(768, 1024)         # q1, k1
            xslice(1024, 1280)
            xslice(1280, 1536)
            wslice(384, 768)          # v chunks 1-3
            xslice(1536, 1792)
            xslice(1792, 2048)
            wslice(1024, 1536)        # q2 k2 q3 k3
            nc.scalar.dma_start(eye[:], eye_d[:])
            nc.scalar.dma_start(woutT[:], woutT_d[:])

            QCOL = [0, 768, 1024, 1280]
            KCOL = [128, 896, 1152, 1408]
            VCOL = 256

            def wcol(cc, col, n):
                return w_sb[:, cc, col : col + n]

            # ---------------- QKV projection pieces ----------------
            def emit_qk_super(jc, t0, n):
                """qk chunk jc (0-3 q, 4-7 k), t range [t0, t0+n)."""
                col = QCOL[jc] if jc < JC else KCOL[jc - JC]
                pq = ps_po.tile([P, 512], F32, tag="po", name=f"qk{jc}_{t0}")
                for cc in range(CC):
                    nc.tensor.matmul(
                        pq[:, 0:n],
                        wcol(cc, col, P),
                        x_sb[:, cc, t0 : t0 + n],
                        start=(cc == 0),
                        stop=(cc == CC - 1),
                    )
                nc.vector.tensor_copy(qkT[:, jc, t0 : t0 + n], pq[:, 0:n])

            def emit_v(lc, ch, nch=1):
                """v chunks [ch, ch+nch) for one 128-token block lc."""
                pq = ps_po.tile([P, 512], F32, tag="po", name=f"v{lc}_{ch}")
                n = nch * P
                for cc in range(CC):
                    nc.tensor.matmul(
                        pq[:, 0:n],
                        x_sb[:, cc, lc * P : (lc + 1) * P],
                        wcol(cc, VCOL + ch * P, n),
                        start=(cc == 0),
                        stop=(cc == CC - 1),
                    )
                nc.vector.tensor_copy(
                    v_aug[:, lc, 2 * ch : 2 * ch + 2 * nch, 0:D],
                    pq[:, 0:n].rearrange("p (h d) -> p h d", d=D),
                )

            def emit_transposes(pair, g):
                """attn_n[i, pair] -> attnT[j, i] for 8 i-blocks of one pair."""
                tp = ps_po.tile([P, HB, P], BF16, tag="po", name=f"tp{pair}_{g}")
                for i in range(HB):
                    ib = g * HB + i
                    nc.tensor.transpose(
                        tp[:, i, :], attn_n[:, ib, pair, :], eye[:]
                    )
                nc.vector.tensor_copy(
                    attnT[:, pair, g * HB * P : (g + 1) * HB * P],
                    tp[:].rearrange("p a b -> p (a b)"),
                )

            # ---------------- output projection block ----------------
            def emit_outproj_block(tb, act_drain=False, tail=False):
                po_ = ps_sc.tile([P, 2, 512], F32, tag="sc", name=f"o_ps{tb}")
                for oc in range(2):
                    for jc in range(JC):
                        nc.tensor.matmul(
                            po_[:, oc, :],
                            attnT[:, jc, tb * P : (tb + 1) * P],
                            woutT[:, jc, oc * 512 : (oc + 1) * 512],
                            start=(jc == 0),
                            stop=(jc == JC - 1),
                        )
                ot = out_pool.tile([P, C], BF16, tag="ot", name=f"ot{tb}")
                # drain in two 512 halves: the psum slot is held until the
                # copy's deferred ack, so one 1024-wide copy would serialize
                # the 3-slot rotation below the PE pace.  act_drain=False
                # keeps ACT free for exp work (threaded blocks).
                nc.vector.tensor_copy(ot[:, 0:512], po_[:, 0, :])
                if act_drain:
                    nc.scalar.copy(ot[:, 512:1024], po_[:, 1, :])
                else:
                    nc.vector.tensor_copy(ot[:, 512:1024], po_[:, 1, :])
                if not tail:
                    nc.sync.dma_start(y_d[tb], ot[:])
                else:
                    # tail: also split the DMA across two queues so the drain
                    # after the last matmul is short
                    nc.sync.dma_start(y_d[tb, :, 0:512], ot[:, 0:512])
                    nc.scalar.dma_start(y_d[tb, :, 512:1024], ot[:, 512:1024])

            # ---------------- insert plan (global item index) ----------------
            # an entry at key k is emitted after item k's PV drain
            inserts = {}
            NITEM = NH * LC

            def add_ins(idx, fn):
                inserts.setdefault(idx, []).append(fn)

            items = [(h, ib) for h in range(NH) for ib in range(LC)]

            # per-item base PE load (cycles): strip of item idx+LEAD + PV
            load = [0.0] * NITEM
            for idx in range(NITEM):
                ib = items[idx][1]
                load[idx] = (ib + 1) * 65 + 120  # PV + drain slack
                if idx + LEAD < NITEM:
                    load[idx] += (items[idx + LEAD][1] + 1) * 128  # strip

            # forced inserts: v chunk 0 block lc before PV(0, lc); placed a
            # few items early so it isn't stuck behind later-x strip supers
            for lc in range(1, LC):
                add_ins(max(lc - 3, 0), (lambda lc=lc: emit_v(lc, 0)))
                load[max(lc - 3, 0)] += CC * P + 120
            # q0/k0 supers are emitted on demand (ensure_bands); account the
            # load at the item whose strip emission pulls them in
            for ib in range(LC):
                load[max(ib - LEAD, 0)] += 2 * CC * P

            # pinned late-stage work: tp(3,0) right after norm(7,0), then the
            # first 6 out-proj token blocks thread into head 7's tail
            add_ins(121, (lambda: emit_transposes(JC - 1, 0)))
            load[121] += HB * P + 120
            for tb in range(6):
                add_ins(122 + tb, (lambda tb=tb: emit_outproj_block(tb)))
                load[122 + tb] += 2 * JC * 512

            # deadline-balanced inserts: (earliest, latest, cycles, fn)
            # earliest indices approximate when the needed x/w DMA has landed
            xband_idx = {0: 0, 1: 1, 2: 3, 3: 5, 4: 7, 5: 8, 6: 9,
                         7: 10, 8: 11}

            def lc_band(lc):
                for bi, (t0, t1) in enumerate(XBANDS):
                    if lc * P < t1:
                        return bi
                return len(XBANDS) - 1

            cand = []
            for ch in (1, 2, 3):
                for lc in range(LC):
                    # v(lc, ch) used by PV(2ch, lc) at item 16*2ch + lc
                    earliest = max(xband_idx[lc_band(lc)], 9)
                    cand.append((earliest, 32 * ch + lc - 1, CC * P + 120,
                                 (lambda lc=lc, ch=ch: emit_v(lc, ch))))
            for pair in range(1, JC):
                w_idx = 6 if pair == 1 else 13
                for tsb in range(NS):
                    earliest = max(xband_idx[tsb + 1], w_idx)
                    latest = 32 * pair + 4 * tsb - LEAD - 1
                    cand.append((earliest, latest, CC * 512,
                                 (lambda p=pair, t=tsb:
                                  emit_qk_super(JC + p, t * 512, 512))))
                    cand.append((earliest, latest, CC * 512,
                                 (lambda p=pair, t=tsb:
                                  emit_qk_super(p, t * 512, 512))))
            for pair in range(JC):
                e0 = 16 * (2 * pair + 1) + 8 + 1   # after norm(2p+1, 0)
                e1 = 16 * (2 * pair + 2)           # after norm(2p+1, 1)
                if pair < JC - 1:
                    cand.append((e0, 118, HB * P + 120,
                                 (lambda p=pair: emit_transposes(p, 0))))
                    cand.append((e1, NITEM - 1, HB * P + 120,
                                 (lambda p=pair: emit_transposes(p, 1))))

            cand.sort(key=lambda e: e[1])
            for earliest, latest, cost, fn in cand:
                lo = max(0, min(earliest, NITEM - 1))
                hi = max(lo, min(latest, NITEM - 1))
                k = min(range(lo, hi + 1), key=lambda i: load[i])
                add_ins(k, fn)
                load[k] += cost

            # ---------------- attention (flat item pipeline) ----------------
            def strip_pieces(ib):
                n = ib + 1
                return [(0, n)] if n <= HB else [(0, HB), (HB, n - HB)]

            def qk_head(h):
                bp = (h % 2) * 64
                chq = h // 2
                return (qkT[bp : bp + 64, chq, :], qkT[bp : bp + 64, JC + chq, :])

            def emit_strip_piece(h, ib, off, n):
                """scores + exp (+ diag mask) for l-blocks [off, off+n) of
                query block ib of head h; returns the bf16 exp strip."""
                qTh, kTh = qk_head(h)
                st = ps_sc.tile([P, HB, P], F32, tag="sc",
                                name=f"st{h}_{ib}_{off}")
                for i in range(n):
                    lb = off + i
                    nc.tensor.matmul(
                        st[:, i, :],
                        kTh[:, lb * P : (lb + 1) * P],
                        qTh[:, ib * P : (ib + 1) * P],
                        start=True, stop=True,
                    )
                exs = exp_pool.tile([P, HB, P], BF16, tag="ex",
                                    name=f"ex{h}_{ib}_{off}")
                nc.scalar.activation(
                    exs[:, 0:n, :].rearrange("p a b -> p (a b)"),
                    st[:, 0:n, :].rearrange("p a b -> p (a b)"),
                    EXPF, scale=SCALE,
                )
                if off <= ib < off + n:
                    nc.gpsimd.tensor_mul(
                        exs[:, ib - off, :], exs[:, ib - off, :], trimask[:]
                    )
                return exs

            def emit_pv_piece(h, ib, po65, exs, off, n):
                for i in range(n):
                    lb = off + i
                    nc.tensor.matmul(
                        po65[:], exs[:, i, :], v_aug[:, lb, h, :],
                        start=(lb == 0), stop=(lb == ib),
                    )

            heads = {}

            def head_tiles(h):
                if h not in heads:
                    attn_raw = dn_pool.tile([P, LC, D + 1], F32, tag="ar",
                                            name=f"ar{h}")
                    dens = dn_pool.tile([P, LC], F32, tag="dn", name=f"dn{h}")
                    recips = dn_pool.tile([P, LC], F32, tag="rc", name=f"rc{h}")
                    rscr = dn_pool.tile([P, LC], F32, tag="rs", name=f"rs{h}")
                    heads[h] = (attn_raw, dens, recips, rscr)
                return heads[h]

            def emit_norm(h, half):
                attn_raw, dens, recips, rscr = head_tiles(h)
                bp = (h % 2) * 64
                chq = h // 2
                s = half * HB
                e = s + HB
                nc.gpsimd.tensor_copy(dens[:, s:e], attn_raw[:, s:e, D])
                nc.vector.reciprocal_approx_accurate(
                    recips[:, s:e], dens[:, s:e], rscr[:, s:e]
                )
                nc.gpsimd.tensor_mul(
                    attn_n[:, s:e, chq, bp : bp + 64],
                    attn_raw[:, s:e, 0:D],
                    recips[:, s:e].broadcast_to((P, HB, D)),
                )

            # q0/k0 supers emitted on demand, one 128-col block at a time,
            # right before the strip needing them: fine granularity avoids
            # head-of-line blocking of the in-order PE queue on the x DMA
            qk0_next = [0]

            def ensure_bands(ib):
                while qk0_next[0] <= ib:
                    tb = qk0_next[0]
                    emit_qk_super(0, tb * P, P)
                    emit_qk_super(JC, tb * P, P)
                    qk0_next[0] += 1

            # prologue: first supers + warmup strips for items 0..LEAD-1
            ensure_bands(0)
            emit_v(0, 0)
            from collections import deque

            strip_q = deque()
            for k in range(LEAD):
                h, ib = items[k]
                if h < 2:
                    ensure_bands(ib)
                strip_q.append(
                    [(emit_strip_piece(h, ib, off, n), off, n)
                     for (off, n) in strip_pieces(ib)]
                )

            for idx, (h, ib) in enumerate(items):
                pieces = strip_q.popleft()
                sp_next = []
                if idx + LEAD < len(items):
                    nh, nib = items[idx + LEAD]
                    if nh < 2:
                        ensure_bands(nib)
                    sp_next = [(nh, nib, off, n) for (off, n) in strip_pieces(nib)]
                po65 = ps_po.tile([P, D + 1], F32, tag="po", name=f"po{h}_{ib}")
                nxt = []
                for j in range(max(len(pieces), len(sp_next))):
                    if j < len(sp_next):
                        nh, nib, off, n = sp_next[j]
                        nxt.append((emit_strip_piece(nh, nib, off, n), off, n))
                    if j < len(pieces):
                        exs, off, n = pieces[j]
                        emit_pv_piece(h, ib, po65, exs, off, n)
                if nxt:
                    strip_q.append(nxt)
                attn_raw = head_tiles(h)[0]
                nc.vector.tensor_copy(attn_raw[:, ib, :], po65[:])
                for fn in inserts.get(idx, []):
                    fn()
                if ib == HB - 1:
                    emit_norm(h, 0)
                elif ib == LC - 1:
                    emit_norm(h, 1)

            # last transpose half can't hide in a later head
            emit_transposes(JC - 1, 1)

            # ---------------- output projection (blocks 6+) ----------------
            for tb in range(6, LC):
                emit_outproj_block(tb, act_drain=True, tail=(tb >= LC - 2))

    nc.compile()
    return nc


_CACHE = {}

# Set by test harnesses to capture a profile; harmless defaults for grading.
TRACE = False
LAST_RESULT = None


def get_program(T=2048):
    if T not in _CACHE:
        _CACHE[T] = build_program(T)
    return _CACHE[T]


def make_in_map(x_b, w_qkv, w_out, hg, T=2048):
    """Host-side shard prep for one core: batch slice x_b [T, C], head group hg."""
    xT = np.ascontiguousarray(x_b.T).astype(NPBF16).reshape(CC, P, T)
    x_t = np.ascontiguousarray(xT.transpose(1, 0, 2))  # [P, CC, T]
    wq = w_qkv[hg * J : (hg + 1) * J]                  # [512, C]
    wk = w_qkv[C + hg * J : C + (hg + 1) * J]
    wv = w_qkv[2 * C + hg * J : 2 * C + (hg + 1) * J]
    # w_sb column order: [q0 k0 | v0 v1 v2 v3 | q1 k1 | q2 k2 | q3 k3]
    W = np.concatenate(
        [wq[0:128], wk[0:128], wv]
        + [np.concatenate([wq[p * 128 : (p + 1) * 128],
                           wk[p * 128 : (p + 1) * 128]])
           for p in range(1, JC)],
        axis=0,
    )  # [3J, C]
    # w_d[p, cc, col] = W[col, cc*128+p]
    wj = np.ascontiguousarray(
        W.T.astype(NPBF16).reshape(CC, P, 3 * J).transpose(1, 0, 2)
    )
    Wo = w_out[:, hg * J : (hg + 1) * J]  # [C, J]
    woutT = np.ascontiguousarray(
        Wo.T.astype(NPBF16).reshape(JC, P, C).transpose(1, 0, 2)
    )
    tri = np.triu(np.ones((P, P), np.float32)).astype(NPBF16)
    eye = np.eye(P, dtype=np.float32).astype(NPBF16)
    return {"wj": wj, "xt": x_t, "woutT": woutT, "trimask": tri, "eye": eye}


def kernel(x, w_qkv, w_out, b_out):
    x = np.asarray(x, dtype=np.float32)
    w_qkv = np.asarray(w_qkv, dtype=np.float32)
    w_out = np.asarray(w_out, dtype=np.float32)
    b_out = np.asarray(b_out, dtype=np.float32)
    B, T, Cx = x.shape
    assert Cx == C

    nc = get_program(T)
    in_maps = [
        make_in_map(x[core // 2], w_qkv, w_out, core % 2, T) for core in range(8)
    ]
    res = run_bass_kernel_spmd(nc, in_maps, core_ids=list(range(8)), trace=TRACE)
    global LAST_RESULT
    LAST_RESULT = res
    outs = [r["y"].astype(np.float32).reshape(T, C) for r in res.results]
    y = np.stack([outs[2 * b] + outs[2 * b + 1] for b in range(B)])
    return (y + b_out[None, None, :]).astype(np.float32)


# revision 51
# speedup vs baseline: 1.1419x; 1.0002x over previous
"""Causal self-attention Bass/Tile kernel for Trainium2, 8 NeuronCores SPMD.

Problem: B=4, T=2048, C=1024, H=16 heads, D=64, f32 in/out.
    qkv = x @ w_qkv.T; per-head causal softmax(q k^T / sqrt(D)) @ v;
    out = attn @ w_out.T + b_out.

Sharding (hybrid batch x tensor-parallel): core c handles batch b = c//2 and
head group hg = c%2 (8 of 16 heads). Each core computes a full [T, C] partial
of the output projection restricted to its heads; the host sums the two
partials per batch (bf16 partials, f32 sum) and adds the bias.

Per-core device algorithm (all matmuls bf16 x bf16 -> f32 PSUM):
  - Inputs arrive as separate w (priority-ordered columns) and x (fine t-band)
    streams on ONE DMA queue so the first score strip issues ~4us in: w[q0],
    x[t<128], w[k0], w[v0] land first, the rest trickles in while head 0
    runs.  Each dma_start costs ~0.6us of descriptor generation, so the
    count is kept low and strictly priority-ordered.
  - qkT is produced in [j, t] layout; chunk-0 supers are emitted on demand
    128 columns at a time right before the strip that needs them (avoids
    head-of-line blocking of the in-order PE queue on the x DMA); later
    chunks go as 512-wide supers.  v is produced in [t, j] layout per
    (block, chunk) with an appended ones column (DVE-broadcast once) for
    the softmax denominators.
  - Attention is a single flat pipeline over items (h, ib) with a LEAD-item
    strip prefetch: at item k the score strip of item k+LEAD is computed
    (k-stationary, <=8-l-block pieces in 2-bank PSUM tiles), exp'd on
    ScalarE (scale=1/8 folded, no max subtraction), causal-masked on the
    diagonal block (GpSimd), while the PV of item k accumulates
    po65[i, 0:65] += ex_strip[lb]^T @ v_aug[lb] into a 1-bank accumulator
    whose 65th column collects the denominators.  The deep lead (10 items,
    22 exs buffers) decouples PV from exp-queue jitter; the binding
    constraint is the 3-slot strip-psum rotation (freed at exp's deferred
    ack).
  - po65 drains raw to SBUF per ib (DVE); normalization is batched per
    half-head on GpSimd (dens gather + broadcast multiply; reciprocal on
    DVE) so the DVE queue never delays po65/qkT drains.
  - attn_n[i, (pair-packed j)] is transposed back to attnT[j, i] with PE
    transpose instructions (bf16 PSUM staging), a head PAIR per [128, 128]
    transpose.
  - QKV supers, v chunks, and transposes are placed by a deadline-balanced
    greedy scheduler (earliest = DMA arrival estimate, latest = first use)
    so per-item PE load stays above the ScalarE exp pace in every head.
  - Output projection from attnT with K=128 chunks; the first 6 token
    blocks thread into head 7's tail; PSUM->SBUF drains are split into two
    512 halves (DVE + ACT) because the psum slot is held until the copy's
    deferred ack; y is stored/DMA'd in BF16 (host sums partials in f32),
    halving the output-DMA tail.

PSUM budget (8 banks): score strips 3x2 (shared with out-proj psum), po65/
filler/transpose rotation 2x1.
"""

import sys

if "/opt/trn_rl_repo" not in sys.path:
    sys.path.insert(0, "/opt/trn_rl_repo")

import numpy as np
import ml_dtypes

import concourse.tile as tile
import concourse.mybir as mybir
from concourse import bacc
from concourse.bass_utils import run_bass_kernel_spmd

BF16 = mybir.dt.bfloat16
F32 = mybir.dt.float32
NPBF16 = ml_dtypes.bfloat16
EXPF = mybir.ActivationFunctionType.Exp

P = 128
C = 1024
CC = C // P      # 8 contraction chunks
NH = 8           # heads per core
D = 64
J = NH * D       # 512 (local q/k/v width)
JC = J // P      # 4 j-chunks

LEAD = 10        # strip prefetch depth (items)


def build_program(T=2048):
    LC = T // P          # l/t 128-blocks
    NS = T // 512        # 512-wide t-supers
    HB = LC // 2         # half-head block count (8)
    SCALE = 0.125        # 1/sqrt(D)

    nc = bacc.Bacc("TRN2", target_bir_lowering=False, debug=False, num_devices=8)

    w_d = nc.dram_tensor("wj", [P, CC, 3 * J], BF16, kind="ExternalInput")
    x_d = nc.dram_tensor("xt", [P, CC, T], BF16, kind="ExternalInput")
    woutT_d = nc.dram_tensor("woutT", [P, JC, C], BF16, kind="ExternalInput")
    mask_d = nc.dram_tensor("trimask", [P, P], BF16, kind="ExternalInput")
    eye_d = nc.dram_tensor("eye", [P, P], BF16, kind="ExternalInput")
    y_d = nc.dram_tensor("y", [LC, P, C], BF16, kind="ExternalOutput")

    # x DMA bands (t ranges); finer bands unlock strip/v work sooner
    XBANDS = [(0, 128), (128, 256), (256, 512), (512, 768), (768, 1024),
              (1024, 1280), (1280, 1536), (1536, 1792), (1792, 2048)]

    with tile.TileContext(nc) as tc:
        with (
            tc.tile_pool(name="persist", bufs=1) as persist,
            tc.tile_pool(name="dn", bufs=2) as dn_pool,
            tc.tile_pool(name="expp", bufs=22) as exp_pool,
            tc.tile_pool(name="outp", bufs=3) as out_pool,
            tc.tile_pool(name="ps_sc", bufs=3, space="PSUM") as ps_sc,
            tc.tile_pool(name="ps_po", bufs=2, space="PSUM") as ps_po,
        ):
            w_sb = persist.tile([P, CC, 3 * J], BF16)
            x_sb = persist.tile([P, CC, T], BF16)
            woutT = persist.tile([P, JC, C], BF16)
            trimask = persist.tile([P, P], BF16)
            eye = persist.tile([P, P], BF16)
            qkT = persist.tile([P, 2 * JC, T], BF16)
            v_aug = persist.tile([P, LC, NH, D + 1], BF16)
            # normalized attention in [i, j] layout; head pair p packs its two
            # heads into one 128-wide slab so a single PE transpose covers both
            attn_n = persist.tile([P, LC, JC, P], BF16)
            attnT = persist.tile([P, JC, T], BF16)

            ones1 = persist.tile([P, 1], BF16)
            nc.vector.memset(ones1[:], 1.0)
            nc.vector.tensor_copy(
                v_aug[:, :, :, D], ones1[:, 0].broadcast_to((P, LC, NH))
            )

            # ---- input DMA stream, ordered for earliest first strip ----
            # w_sb column order: [q0 k0 | v0..v3 | q1 k1 | q2 k2 | q3 k3] so
            # priority ranges are contiguous.  One queue = strict DMA order;
            # each dma_start costs ~0.6us of descriptor generation, so keep
            # the count low and front-load only what unblocks compute.
            def wslice(c0, c1):
                nc.sync.dma_start(w_sb[:, :, c0:c1], w_d[:, :, c0:c1])

            def xslice(t0, t1):
                nc.sync.dma_start(x_sb[:, :, t0:t1], x_d[:, :, t0:t1])

            nc.scalar.dma_start(trimask[:], mask_d[:])
            wslice(0, 128)            # q0
            xslice(0, 128)
            wslice(128, 256)          # k0
            wslice(256, 384)          # v chunk 0 (heads 0-1)
            xslice(128, 256)
            xslice(256, 512)
            xslice(512, 768)
            xslice(768, 1024)
            wslice(768, 1024)         # q1, k1
            xslice(1024, 1280)
            xslice(1280, 1536)
            wslice(384, 768)          # v chunks 1-3
            xslice(1536, 1792)
            xslice(1792, 2048)
            wslice(1024, 1536)        # q2 k2 q3 k3
            nc.scalar.dma_start(eye[:], eye_d[:])
            nc.scalar.dma_start(woutT[:], woutT_d[:])

            QCOL = [0, 768, 1024, 1280]
            KCOL = [128, 896, 1152, 1408]
            VCOL = 256

            def wcol(cc, col, n):
                return w_sb[:, cc, col : col + n]

            # ---------------- QKV projection pieces ----------------
            def emit_qk_super(jc, t0, n):
                """qk chunk jc (0-3 q, 4-7 k), t range [t0, t0+n)."""
                col = QCOL[jc] if jc < JC else KCOL[jc - JC]
                pq = ps_po.tile([P, 512], F32, tag="po", name=f"qk{jc}_{t0}")
                for cc in range(CC):
                    nc.tensor.matmul(
                        pq[:, 0:n],
                        wcol(cc, col, P),
                        x_sb[:, cc, t0 : t0 + n],
                        start=(cc == 0),
                        stop=(cc == CC - 1),
                    )
                nc.vector.tensor_copy(qkT[:, jc, t0 : t0 + n], pq[:, 0:n])

            def emit_v(lc, ch, nch=1):
                """v chunks [ch, ch+nch) for one 128-token block lc."""
                pq = ps_po.tile([P, 512], F32, tag="po", name=f"v{lc}_{ch}")
                n = nch * P
                for cc in range(CC):
                    nc.tensor.matmul(
                        pq[:, 0:n],
                        x_sb[:, cc, lc * P : (lc + 1) * P],
                        wcol(cc, VCOL + ch * P, n),
                        start=(cc == 0),
                        stop=(cc == CC - 1),
                    )
                nc.vector.tensor_copy(
                    v_aug[:, lc, 2 * ch : 2 * ch + 2 * nch, 0:D],
                    pq[:, 0:n].rearrange("p (h d) -> p h d", d=D),
                )

            def emit_transposes(pair, g):
                """attn_n[i, pair] -> attnT[j, i] for 8 i-blocks of one pair."""
                tp = ps_po.tile([P, HB, P], BF16, tag="po", name=f"tp{pair}_{g}")
                for i in range(HB):
                    ib = g * HB + i
                    nc.tensor.transpose(
                        tp[:, i, :], attn_n[:, ib, pair, :], eye[:]
                    )
                nc.vector.tensor_copy(
                    attnT[:, pair, g * HB * P : (g + 1) * HB * P],
                    tp[:].rearrange("p a b -> p (a b)"),
                )

            # ---------------- output projection block ----------------
            def emit_outproj_block(tb, act_drain=False, tail=False):
                po_ = ps_sc.tile([P, 2, 512], F32, tag="sc", name=f"o_ps{tb}")
                for oc in range(2):
                    for jc in range(JC):
                        nc.tensor.matmul(
                            po_[:, oc, :],
                            attnT[:, jc, tb * P : (tb + 1) * P],
                            woutT[:, jc, oc * 512 : (oc + 1) * 512],
                            start=(jc == 0),
                            stop=(jc == JC - 1),
                        )
                ot = out_pool.tile([P, C], BF16, tag="ot", name=f"ot{tb}")
                # drain in two 512 halves: the psum slot is held until the
                # copy's deferred ack, so one 1024-wide copy would serialize
                # the 3-slot rotation below the PE pace.  act_drain=False
                # keeps ACT free for exp work (threaded blocks).
                nc.vector.tensor_copy(ot[:, 0:512], po_[:, 0, :])
                if act_drain:
                    nc.scalar.copy(ot[:, 512:1024], po_[:, 1, :])
                else:
                    nc.vector.tensor_copy(ot[:, 512:1024], po_[:, 1, :])
                if not tail:
                    nc.sync.dma_start(y_d[tb], ot[:])
                else:
                    # tail: also split the DMA across two queues so the drain
                    # after the last matmul is short
                    nc.sync.dma_start(y_d[tb, :, 0:512], ot[:, 0:512])
                    nc.scalar.dma_start(y_d[tb, :, 512:1024], ot[:, 512:1024])

            # ---------------- insert plan (global item index) ----------------
            # an entry at key k is emitted after item k's PV drain
            inserts = {}
            NITEM = NH * LC

            def add_ins(idx, fn):
                inserts.setdefault(idx, []).append(fn)

            items = [(h, ib) for h in range(NH) for ib in range(LC)]

            # per-item base PE load (cycles): strip of item idx+LEAD + PV
            load = [0.0] * NITEM
            for idx in range(NITEM):
                ib = items[idx][1]
                load[idx] = (ib + 1) * 65 + 120  # PV + drain slack
                if idx + LEAD < NITEM:
                    load[idx] += (items[idx + LEAD][1] + 1) * 128  # strip

            # forced inserts: v chunk 0 block lc before PV(0, lc); placed a
            # few items early so it isn't stuck behind later-x strip supers
            for lc in range(1, LC):
                add_ins(max(lc - 3, 0), (lambda lc=lc: emit_v(lc, 0)))
                load[max(lc - 3, 0)] += CC * P + 120
            # q0/k0 supers are emitted on demand (ensure_bands); account the
            # load at the item whose strip emission pulls them in
            for ib in range(LC):
                load[max(ib - LEAD, 0)] += 2 * CC * P

            # pinned late-stage work: tp(3,0) right after norm(7,0), then the
            # first 6 out-proj token blocks thread into head 7's tail
            add_ins(121, (lambda: emit_transposes(JC - 1, 0)))
            load[121] += HB * P + 120
            for tb in range(6):
                add_ins(122 + tb, (lambda tb=tb: emit_outproj_block(tb)))
                load[122 + tb] += 2 * JC * 512

            # deadline-balanced inserts: (earliest, latest, cycles, fn)
            # earliest indices approximate when the needed x/w DMA has landed
            xband_idx = {0: 0, 1: 1, 2: 3, 3: 5, 4: 7, 5: 8, 6: 9,
                         7: 10, 8: 11}

            def lc_band(lc):
                for bi, (t0, t1) in enumerate(XBANDS):
                    if lc * P < t1:
                        return bi
                return len(XBANDS) - 1

            cand = []
            for ch in (1, 2, 3):
                for lc in range(LC):
                    # v(lc, ch) used by PV(2ch, lc) at item 16*2ch + lc
                    earliest = max(xband_idx[lc_band(lc)], 7)
                    cand.append((earliest, 32 * ch + lc - 1, CC * P + 120,
                                 (lambda lc=lc, ch=ch: emit_v(lc, ch))))
            for pair in range(1, JC):
                w_idx = 6 if pair == 1 else 13
                for tsb in range(NS):
                    earliest = max(xband_idx[tsb + 1], w_idx)
                    latest = 32 * pair + 4 * tsb - LEAD - 1
                    cand.append((earliest, latest, CC * 512,
                                 (lambda p=pair, t=tsb:
                                  emit_qk_super(JC + p, t * 512, 512))))
                    cand.append((earliest, latest, CC * 512,
                                 (lambda p=pair, t=tsb:
                                  emit_qk_super(p, t * 512, 512))))
            for pair in range(JC):
                e0 = 16 * (2 * pair + 1) + 8 + 1   # after norm(2p+1, 0)
                e1 = 16 * (2 * pair + 2)           # after norm(2p+1, 1)
                if pair < JC - 1:
                    cand.append((e0, 118, HB * P + 120,
                                 (lambda p=pair: emit_transposes(p, 0))))
                    cand.append((e1, NITEM - 1, HB * P + 120,
                                 (lambda p=pair: emit_transposes(p, 1))))

            cand.sort(key=lambda e: e[1])
            for earliest, latest, cost, fn in cand:
                lo = max(0, min(earliest, NITEM - 1))
                hi = max(lo, min(latest, NITEM - 1))
                k = min(range(lo, hi + 1), key=lambda i: load[i])
                add_ins(k, fn)
                load[k] += cost

            # ---------------- attention (flat item pipeline) ----------------
            def strip_pieces(ib):
                n = ib + 1
                return [(0, n)] if n <= HB else [(0, HB), (HB, n - HB)]

            def qk_head(h):
                bp = (h % 2) * 64
                chq = h // 2
                return (qkT[bp : bp + 64, chq, :], qkT[bp : bp + 64, JC + chq, :])

            def emit_strip_piece(h, ib, off, n):
                """scores + exp (+ diag mask) for l-blocks [off, off+n) of
                query block ib of head h; returns the bf16 exp strip."""
                qTh, kTh = qk_head(h)
                st = ps_sc.tile([P, HB, P], F32, tag="sc",
                                name=f"st{h}_{ib}_{off}")
                for i in range(n):
                    lb = off + i
                    nc.tensor.matmul(
                        st[:, i, :],
                        kTh[:, lb * P : (lb + 1) * P],
                        qTh[:, ib * P : (ib + 1) * P],
                        start=True, stop=True,
                    )
                exs = exp_pool.tile([P, HB, P], BF16, tag="ex",
                                    name=f"ex{h}_{ib}_{off}")
                nc.scalar.activation(
                    exs[:, 0:n, :].rearrange("p a b -> p (a b)"),
                    st[:, 0:n, :].rearrange("p a b -> p (a b)"),
                    EXPF, scale=SCALE,
                )
                if off <= ib < off + n:
                    nc.gpsimd.tensor_mul(
                        exs[:, ib - off, :], exs[:, ib - off, :], trimask[:]
                    )
                return exs

            def emit_pv_piece(h, ib, po65, exs, off, n):
                for i in range(n):
                    lb = off + i
                    nc.tensor.matmul(
                        po65[:], exs[:, i, :], v_aug[:, lb, h, :],
                        start=(lb == 0), stop=(lb == ib),
                    )

            heads = {}

            def head_tiles(h):
                if h not in heads:
                    attn_raw = dn_pool.tile([P, LC, D + 1], F32, tag="ar",
                                            name=f"ar{h}")
                    dens = dn_pool.tile([P, LC], F32, tag="dn", name=f"dn{h}")
                    recips = dn_pool.tile([P, LC], F32, tag="rc", name=f"rc{h}")
                    rscr = dn_pool.tile([P, LC], F32, tag="rs", name=f"rs{h}")
                    heads[h] = (attn_raw, dens, recips, rscr)
                return heads[h]

            def emit_norm(h, half):
                attn_raw, dens, recips, rscr = head_tiles(h)
                bp = (h % 2) * 64
                chq = h // 2
                s = half * HB
                e = s + HB
                nc.gpsimd.tensor_copy(dens[:, s:e], attn_raw[:, s:e, D])
                nc.vector.reciprocal_approx_accurate(
                    recips[:, s:e], dens[:, s:e], rscr[:, s:e]
                )
                nc.gpsimd.tensor_mul(
                    attn_n[:, s:e, chq, bp : bp + 64],
                    attn_raw[:, s:e, 0:D],
                    recips[:, s:e].broadcast_to((P, HB, D)),
                )

            # q0/k0 supers emitted on demand, one 128-col block at a time,
            # right before the strip needing them: fine granularity avoids
            # head-of-line blocking of the in-order PE queue on the x DMA
            qk0_next = [0]

            def ensure_bands(ib):
                while qk0_next[0] <= ib:
                    tb = qk0_next[0]
                    emit_qk_super(0, tb * P, P)
                    emit_qk_super(JC, tb * P, P)
                    qk0_next[0] += 1

            # prologue: first supers + warmup strips for items 0..LEAD-1
            ensure_bands(0)
            emit_v(0, 0)
            from collections import deque

            strip_q = deque()
            for k in range(LEAD):
                h, ib = items[k]
                if h < 2:
                    ensure_bands(ib)
                strip_q.append(
                    [(emit_strip_piece(h, ib, off, n), off, n)
                     for (off, n) in strip_pieces(ib)]
                )

            for idx, (h, ib) in enumerate(items):
                pieces = strip_q.popleft()
                sp_next = []
                if idx + LEAD < len(items):
                    nh, nib = items[idx + LEAD]
                    if nh < 2:
                        ensure_bands(nib)
                    sp_next = [(nh, nib, off, n) for (off, n) in strip_pieces(nib)]
                po65 = ps_po.tile([P, D + 1], F32, tag="po", name=f"po{h}_{ib}")
                nxt = []
                for j in range(max(len(pieces), len(sp_next))):
                    if j < len(sp_next):
                        nh, nib, off, n = sp_next[j]
                        nxt.append((emit_strip_piece(nh, nib, off, n), off, n))
                    if j < len(pieces):
                        exs, off, n = pieces[j]
                        emit_pv_piece(h, ib, po65, exs, off, n)
                if nxt:
                    strip_q.append(nxt)
                attn_raw = head_tiles(h)[0]
                nc.vector.tensor_copy(attn_raw[:, ib, :], po65[:])
                for fn in inserts.get(idx, []):
                    fn()
                if ib == HB - 1:
                    emit_norm(h, 0)
                elif ib == LC - 1:
                    emit_norm(h, 1)

            # last transpose half can't hide in a later head
            emit_transposes(JC - 1, 1)

            # ---------------- output projection (blocks 6+) ----------------
            for tb in range(6, LC):
                emit_outproj_block(tb, act_drain=True, tail=(tb >= LC - 2))

    nc.compile()
    return nc


_CACHE = {}

# Set by test harnesses to capture a profile; harmless defaults for grading.
TRACE = False
LAST_RESULT = None


def get_program(T=2048):
    if T not in _CACHE:
        _CACHE[T] = build_program(T)
    return _CACHE[T]


def make_in_map(x_b, w_qkv, w_out, hg, T=2048):
    """Host-side shard prep for one core: batch slice x_b [T, C], head group hg."""
    xT = np.ascontiguousarray(x_b.T).astype(NPBF16).reshape(CC, P, T)
    x_t = np.ascontiguousarray(xT.transpose(1, 0, 2))  # [P, CC, T]
    wq = w_qkv[hg * J : (hg + 1) * J]                  # [512, C]
    wk = w_qkv[C + hg * J : C + (hg + 1) * J]
    wv = w_qkv[2 * C + hg * J : 2 * C + (hg + 1) * J]
    # w_sb column order: [q0 k0 | v0 v1 v2 v3 | q1 k1 | q2 k2 | q3 k3]
    W = np.concatenate(
        [wq[0:128], wk[0:128], wv]
        + [np.concatenate([wq[p * 128 : (p + 1) * 128],
                           wk[p * 128 : (p + 1) * 128]])
           for p in range(1, JC)],
        axis=0,
    )  # [3J, C]
    # w_d[p, cc, col] = W[col, cc*128+p]
    wj = np.ascontiguousarray(
        W.T.astype(NPBF16).reshape(CC, P, 3 * J).transpose(1, 0, 2)
    )
    Wo = w_out[:, hg * J : (hg + 1) * J]  # [C, J]
    woutT = np.ascontiguousarray(
        Wo.T.astype(NPBF16).reshape(JC, P, C).transpose(1, 0, 2)
    )
    tri = np.triu(np.ones((P, P), np.float32)).astype(NPBF16)
    eye = np.eye(P, dtype=np.float32).astype(NPBF16)
    return {"wj": wj, "xt": x_t, "woutT": woutT, "trimask": tri, "eye": eye}


def kernel(x, w_qkv, w_out, b_out):
    x = np.asarray(x, dtype=np.float32)
    w_qkv = np.asarray(w_qkv, dtype=np.float32)
    w_out = np.asarray(w_out, dtype=np.float32)
    b_out = np.asarray(b_out, dtype=np.float32)
    B, T, Cx = x.shape
    assert Cx == C

    nc = get_program(T)
    in_maps = [
        make_in_map(x[core // 2], w_qkv, w_out, core % 2, T) for core in range(8)
    ]
    res = run_bass_kernel_spmd(nc, in_maps, core_ids=list(range(8)), trace=TRACE)
    global LAST_RESULT
    LAST_RESULT = res
    outs = [r["y"].astype(np.float32).reshape(T, C) for r in res.results]
    y = np.stack([outs[2 * b] + outs[2 * b + 1] for b in range(B)])
    return (y + b_out[None, None, :]).astype(np.float32)


# revision 66
# speedup vs baseline: 1.1458x; 1.0034x over previous
"""Causal self-attention Bass/Tile kernel for Trainium2, 8 NeuronCores SPMD.

Problem: B=4, T=2048, C=1024, H=16 heads, D=64, f32 in/out.
    qkv = x @ w_qkv.T; per-head causal softmax(q k^T / sqrt(D)) @ v;
    out = attn @ w_out.T + b_out.

Sharding (hybrid batch x tensor-parallel): core c handles batch b = c//2 and
head group hg = c%2 (8 of 16 heads). Each core computes a full [T, C] partial
of the output projection restricted to its heads; the host sums the two
partials per batch (bf16 partials, f32 sum) and adds the bias.

Per-core device algorithm (all matmuls bf16 x bf16 -> f32 PSUM):
  - Inputs arrive as separate w (priority-ordered columns) and x (fine t-band)
    streams on ONE DMA queue so the first score strip issues ~4us in: w[q0],
    x[t<128], w[k0], w[v0] land first, the rest trickles in while head 0
    runs.  Each dma_start costs ~0.6us of descriptor generation, so the
    count is kept low and strictly priority-ordered.
  - qkT is produced in [j, t] layout; chunk-0 supers are emitted on demand
    128 columns at a time right before the strip that needs them (avoids
    head-of-line blocking of the in-order PE queue on the x DMA); later
    chunks go as 512-wide supers.  v is produced in [t, j] layout per
    (block, chunk) with an appended ones column (DVE-broadcast once) for
    the softmax denominators.
  - Attention is a single flat pipeline over items (h, ib) with a ramped
    strip prefetch: score strips are computed ahead of their PV consumption
    (k-stationary, <=8-l-block pieces in 2-bank PSUM tiles), exp'd on
    ScalarE (scale=1/8 folded, no max subtraction), causal-masked on the
    diagonal block (GpSimd), while the PV of item k accumulates
    po65[i, 0:65] += ex_strip[lb]^T @ v_aug[lb] into a 1-bank accumulator
    whose 65th column collects the denominators.  The lead ramps 8 -> 10
    items (22 exs buffers): deep enough to decouple PV from exp-queue
    jitter, shallow enough at the start that head-0 strips don't
    head-of-line-block the in-order PE queue on unarrived x bands; the
    binding constraint is the 3-slot strip-psum rotation (freed at exp's
    deferred ack).
  - po65 drains raw to SBUF per ib (DVE); normalization is batched per
    half-head on GpSimd (dens gather + broadcast multiply; reciprocal on
    DVE) so the DVE queue never delays po65/qkT drains.
  - attn_n[i, (pair-packed j)] is transposed back to attnT[j, i] with PE
    transpose instructions (bf16 PSUM staging), a head PAIR per [128, 128]
    transpose.
  - QKV supers, v chunks, and transposes are placed by a deadline-balanced
    greedy scheduler (earliest = DMA arrival estimate, latest = first use)
    so per-item PE load stays above the ScalarE exp pace in every head.
  - Output projection from attnT with K=128 chunks; the first 6 token
    blocks thread into head 7's tail; PSUM->SBUF drains are split into two
    512 halves (DVE + ACT) because the psum slot is held until the copy's
    deferred ack; y is stored/DMA'd in BF16 (host sums partials in f32),
    halving the output-DMA tail.

PSUM budget (8 banks): score strips 3x2 (shared with out-proj psum), po65/
filler/transpose rotation 2x1.
"""

import sys

if "/opt/trn_rl_repo" not in sys.path:
    sys.path.insert(0, "/opt/trn_rl_repo")

import numpy as np
import ml_dtypes

import concourse.tile as tile
import concourse.mybir as mybir
from concourse import bacc
from concourse.bass_utils import run_bass_kernel_spmd

BF16 = mybir.dt.bfloat16
F32 = mybir.dt.float32
NPBF16 = ml_dtypes.bfloat16
EXPF = mybir.ActivationFunctionType.Exp

P = 128
C = 1024
CC = C // P      # 8 contraction chunks
NH = 8           # heads per core
D = 64
J = NH * D       # 512 (local q/k/v width)
JC = J // P      # 4 j-chunks

LEAD = 10        # strip prefetch depth (items)


def build_program(T=2048):
    LC = T // P          # l/t 128-blocks
    NS = T // 512        # 512-wide t-supers
    HB = LC // 2         # half-head block count (8)
    SCALE = 0.125        # 1/sqrt(D)

    nc = bacc.Bacc("TRN2", target_bir_lowering=False, debug=False, num_devices=8)

    w_d = nc.dram_tensor("wj", [P, CC, 3 * J], BF16, kind="ExternalInput")
    x_d = nc.dram_tensor("xt", [P, CC, T], BF16, kind="ExternalInput")
    woutT_d = nc.dram_tensor("woutT", [P, JC, C], BF16, kind="ExternalInput")
    mask_d = nc.dram_tensor("trimask", [P, P], BF16, kind="ExternalInput")
    eye_d = nc.dram_tensor("eye", [P, P], BF16, kind="ExternalInput")
    y_d = nc.dram_tensor("y", [LC, P, C], BF16, kind="ExternalOutput")

    # x DMA bands (t ranges); finer bands unlock strip/v work sooner
    XBANDS = [(0, 128), (128, 256), (256, 512), (512, 768), (768, 1024),
              (1024, 1280), (1280, 1536), (1536, 1792), (1792, 2048)]

    with tile.TileContext(nc) as tc:
        with (
            tc.tile_pool(name="persist", bufs=1) as persist,
            tc.tile_pool(name="dn", bufs=2) as dn_pool,
            tc.tile_pool(name="expp", bufs=22) as exp_pool,
            tc.tile_pool(name="outp", bufs=4) as out_pool,
            tc.tile_pool(name="ps_sc", bufs=3, space="PSUM") as ps_sc,
            tc.tile_pool(name="ps_po", bufs=2, space="PSUM") as ps_po,
        ):
            w_sb = persist.tile([P, CC, 3 * J], BF16)
            x_sb = persist.tile([P, CC, T], BF16)
            woutT = persist.tile([P, JC, C], BF16)
            trimask = persist.tile([P, P], BF16)
            eye = persist.tile([P, P], BF16)
            qkT = persist.tile([P, 2 * JC, T], BF16)
            v_aug = persist.tile([P, LC, NH, D + 1], BF16)
            # normalized attention in [i, j] layout; head pair p packs its two
            # heads into one 128-wide slab so a single PE transpose covers both
            attn_n = persist.tile([P, LC, JC, P], BF16)
            attnT = persist.tile([P, JC, T], BF16)

            ones1 = persist.tile([P, 1], BF16)
            nc.vector.memset(ones1[:], 1.0)
            nc.vector.tensor_copy(
                v_aug[:, :, :, D], ones1[:, 0].broadcast_to((P, LC, NH))
            )

            # ---- input DMA stream, ordered for earliest first strip ----
            # w_sb column order: [q0 k0 | v0..v3 | q1 k1 | q2 k2 | q3 k3] so
            # priority ranges are contiguous.  One queue = strict DMA order;
            # each dma_start costs ~0.6us of descriptor generation, so keep
            # the count low and front-load only what unblocks compute.
            def wslice(c0, c1):
                nc.sync.dma_start(w_sb[:, :, c0:c1], w_d[:, :, c0:c1])

            def xslice(t0, t1):
                nc.sync.dma_start(x_sb[:, :, t0:t1], x_d[:, :, t0:t1])

            wslice(0, 128)            # q0
            xslice(0, 128)
            wslice(128, 256)          # k0
            nc.scalar.dma_start(trimask[:], mask_d[:])
            wslice(256, 384)          # v chunk 0 (heads 0-1)
            xslice(128, 256)
            xslice(256, 512)
            xslice(512, 768)
            xslice(768, 1024)
            wslice(768, 1024)         # q1, k1
            xslice(1024, 1280)
            xslice(1280, 1536)
            wslice(384, 768)          # v chunks 1-3
            xslice(1536, 1792)
            xslice(1792, 2048)
            wslice(1024, 1536)        # q2 k2 q3 k3
            nc.scalar.dma_start(eye[:], eye_d[:])
            nc.scalar.dma_start(woutT[:], woutT_d[:])

            QCOL = [0, 768, 1024, 1280]
            KCOL = [128, 896, 1152, 1408]
            VCOL = 256

            def wcol(cc, col, n):
                return w_sb[:, cc, col : col + n]

            # ---------------- QKV projection pieces ----------------
            def emit_qk_super(jc, t0, n):
                """qk chunk jc (0-3 q, 4-7 k), t range [t0, t0+n)."""
                col = QCOL[jc] if jc < JC else KCOL[jc - JC]
                pq = ps_po.tile([P, 512], F32, tag="po", name=f"qk{jc}_{t0}")
                for cc in range(CC):
                    nc.tensor.matmul(
                        pq[:, 0:n],
                        wcol(cc, col, P),
                        x_sb[:, cc, t0 : t0 + n],
                        start=(cc == 0),
                        stop=(cc == CC - 1),
                    )
                nc.vector.tensor_copy(qkT[:, jc, t0 : t0 + n], pq[:, 0:n])

            def emit_v(lc, ch, nch=1):
                """v chunks [ch, ch+nch) for one 128-token block lc."""
                pq = ps_po.tile([P, 512], F32, tag="po", name=f"v{lc}_{ch}")
                n = nch * P
                for cc in range(CC):
                    nc.tensor.matmul(
                        pq[:, 0:n],
                        x_sb[:, cc, lc * P : (lc + 1) * P],
                        wcol(cc, VCOL + ch * P, n),
                        start=(cc == 0),
                        stop=(cc == CC - 1),
                    )
                nc.vector.tensor_copy(
                    v_aug[:, lc, 2 * ch : 2 * ch + 2 * nch, 0:D],
                    pq[:, 0:n].rearrange("p (h d) -> p h d", d=D),
                )

            def emit_transposes(pair, g):
                """attn_n[i, pair] -> attnT[j, i] for 8 i-blocks of one pair."""
                tp = ps_po.tile([P, HB, P], BF16, tag="po", name=f"tp{pair}_{g}")
                for i in range(HB):
                    ib = g * HB + i
                    nc.tensor.transpose(
                        tp[:, i, :], attn_n[:, ib, pair, :], eye[:]
                    )
                nc.vector.tensor_copy(
                    attnT[:, pair, g * HB * P : (g + 1) * HB * P],
                    tp[:].rearrange("p a b -> p (a b)"),
                )

            # ---------------- output projection block ----------------
            def emit_outproj_block(tb, act_drain=False, tail=False):
                po_ = ps_sc.tile([P, 2, 512], F32, tag="sc", name=f"o_ps{tb}")
                for oc in range(2):
                    for jc in range(JC):
                        nc.tensor.matmul(
                            po_[:, oc, :],
                            attnT[:, jc, tb * P : (tb + 1) * P],
                            woutT[:, jc, oc * 512 : (oc + 1) * 512],
                            start=(jc == 0),
                            stop=(jc == JC - 1),
                        )
                ot = out_pool.tile([P, C], BF16, tag="ot", name=f"ot{tb}")
                # drain in two 512 halves: the psum slot is held until the
                # copy's deferred ack, so one 1024-wide copy would serialize
                # the 3-slot rotation below the PE pace.  act_drain=False
                # keeps ACT free for exp work (threaded blocks).
                nc.vector.tensor_copy(ot[:, 0:512], po_[:, 0, :])
                if act_drain:
                    nc.scalar.copy(ot[:, 512:1024], po_[:, 1, :])
                else:
                    nc.vector.tensor_copy(ot[:, 512:1024], po_[:, 1, :])
                if not tail:
                    nc.sync.dma_start(y_d[tb], ot[:])
                else:
                    # tail: also split the DMA across two queues so the drain
                    # after the last matmul is short
                    nc.sync.dma_start(y_d[tb, :, 0:512], ot[:, 0:512])
                    nc.scalar.dma_start(y_d[tb, :, 512:1024], ot[:, 512:1024])

            # ---------------- insert plan (global item index) ----------------
            # an entry at key k is emitted after item k's PV drain
            inserts = {}
            NITEM = NH * LC

            def add_ins(idx, fn):
                inserts.setdefault(idx, []).append(fn)

            items = [(h, ib) for h in range(NH) for ib in range(LC)]

            # per-item base PE load (cycles): strip of item idx+LEAD + PV
            load = [0.0] * NITEM
            for idx in range(NITEM):
                ib = items[idx][1]
                load[idx] = (ib + 1) * 65 + 120  # PV + drain slack
                if idx + LEAD < NITEM:
                    load[idx] += (items[idx + LEAD][1] + 1) * 128  # strip

            # forced inserts: v chunk 0 block lc before PV(0, lc); placed a
            # few items early so it isn't stuck behind later-x strip supers
            for lc in range(1, LC):
                add_ins(max(lc - 3, 0), (lambda lc=lc: emit_v(lc, 0)))
                load[max(lc - 3, 0)] += CC * P + 120
            # q0/k0 supers are emitted on demand (ensure_bands); account the
            # load at the item whose strip emission pulls them in
            for ib in range(LC):
                load[max(ib - LEAD, 0)] += 2 * CC * P

            # pinned late-stage work: tp(3,0) right after norm(7,0), then the
            # first 6 out-proj token blocks thread into head 7's tail
            add_ins(121, (lambda: emit_transposes(JC - 1, 0)))
            load[121] += HB * P + 120
            for tb in range(6):
                add_ins(122 + tb, (lambda tb=tb: emit_outproj_block(tb)))
                load[122 + tb] += 2 * JC * 512

            # deadline-balanced inserts: (earliest, latest, cycles, fn)
            # earliest indices approximate when the needed x/w DMA has landed
            xband_idx = {0: 0, 1: 1, 2: 3, 3: 5, 4: 7, 5: 8, 6: 9,
                         7: 10, 8: 11}

            def lc_band(lc):
                for bi, (t0, t1) in enumerate(XBANDS):
                    if lc * P < t1:
                        return bi
                return len(XBANDS) - 1

            cand = []
            for ch in (1, 2, 3):
                for lc in range(LC):
                    # v(lc, ch) used by PV(2ch, lc) at item 16*2ch + lc
                    earliest = max(xband_idx[lc_band(lc)], 7)
                    cand.append((earliest, 32 * ch + lc - 1, CC * P + 120,
                                 (lambda lc=lc, ch=ch: emit_v(lc, ch))))
            for pair in range(1, JC):
                w_idx = 6 if pair == 1 else 13
                for tsb in range(NS):
                    earliest = max(xband_idx[tsb + 1], w_idx)
                    latest = 32 * pair + 4 * tsb - LEAD - 1
                    cand.append((earliest, latest, CC * 512,
                                 (lambda p=pair, t=tsb:
                                  emit_qk_super(JC + p, t * 512, 512))))
                    cand.append((earliest, latest, CC * 512,
                                 (lambda p=pair, t=tsb:
                                  emit_qk_super(p, t * 512, 512))))
            for pair in range(JC):
                e0 = 16 * (2 * pair + 1) + 8 + 1   # after norm(2p+1, 0)
                e1 = 16 * (2 * pair + 2)           # after norm(2p+1, 1)
                if pair < JC - 1:
                    cand.append((e0, 118, HB * P + 120,
                                 (lambda p=pair: emit_transposes(p, 0))))
                    cand.append((e1, NITEM - 1, HB * P + 120,
                                 (lambda p=pair: emit_transposes(p, 1))))

            cand.sort(key=lambda e: e[1])
            for earliest, latest, cost, fn in cand:
                lo = max(0, min(earliest, NITEM - 1))
                hi = max(lo, min(latest, NITEM - 1))
                k = min(range(lo, hi + 1), key=lambda i: load[i])
                add_ins(k, fn)
                load[k] += cost

            # ---------------- attention (flat item pipeline) ----------------
            def strip_pieces(ib):
                n = ib + 1
                return [(0, n)] if n <= HB else [(0, HB), (HB, n - HB)]

            def qk_head(h):
                bp = (h % 2) * 64
                chq = h // 2
                return (qkT[bp : bp + 64, chq, :], qkT[bp : bp + 64, JC + chq, :])

            def emit_strip_piece(h, ib, off, n):
                """scores + exp (+ diag mask) for l-blocks [off, off+n) of
                query block ib of head h; returns the bf16 exp strip."""
                qTh, kTh = qk_head(h)
                st = ps_sc.tile([P, HB, P], F32, tag="sc",
                                name=f"st{h}_{ib}_{off}")
                for i in range(n):
                    lb = off + i
                    nc.tensor.matmul(
                        st[:, i, :],
                        kTh[:, lb * P : (lb + 1) * P],
                        qTh[:, ib * P : (ib + 1) * P],
                        start=True, stop=True,
                    )
                exs = exp_pool.tile([P, HB, P], BF16, tag="ex",
                                    name=f"ex{h}_{ib}_{off}")
                nc.scalar.activation(
                    exs[:, 0:n, :].rearrange("p a b -> p (a b)"),
                    st[:, 0:n, :].rearrange("p a b -> p (a b)"),
                    EXPF, scale=SCALE,
                )
                if off <= ib < off + n:
                    nc.gpsimd.tensor_mul(
                        exs[:, ib - off, :], exs[:, ib - off, :], trimask[:]
                    )
                return exs

            def emit_pv_piece(h, ib, po65, exs, off, n):
                for i in range(n):
                    lb = off + i
                    nc.tensor.matmul(
                        po65[:], exs[:, i, :], v_aug[:, lb, h, :],
                        start=(lb == 0), stop=(lb == ib),
                    )

            heads = {}

            def head_tiles(h):
                if h not in heads:
                    attn_raw = dn_pool.tile([P, LC, D + 1], F32, tag="ar",
                                            name=f"ar{h}")
                    dens = dn_pool.tile([P, LC], F32, tag="dn", name=f"dn{h}")
                    recips = dn_pool.tile([P, LC], F32, tag="rc", name=f"rc{h}")
                    rscr = dn_pool.tile([P, LC], F32, tag="rs", name=f"rs{h}")
                    heads[h] = (attn_raw, dens, recips, rscr)
                return heads[h]

            def emit_norm(h, half):
                attn_raw, dens, recips, rscr = head_tiles(h)
                bp = (h % 2) * 64
                chq = h // 2
                s = half * HB
                e = s + HB
                nc.gpsimd.tensor_copy(dens[:, s:e], attn_raw[:, s:e, D])
                nc.vector.reciprocal_approx_accurate(
                    recips[:, s:e], dens[:, s:e], rscr[:, s:e]
                )
                nc.gpsimd.tensor_mul(
                    attn_n[:, s:e, chq, bp : bp + 64],
                    attn_raw[:, s:e, 0:D],
                    recips[:, s:e].broadcast_to((P, HB, D)),
                )

            # q0/k0 supers emitted on demand, one 128-col block at a time,
            # right before the strip needing them: fine granularity avoids
            # head-of-line blocking of the in-order PE queue on the x DMA
            qk0_next = [0]

            def ensure_bands(ib):
                while qk0_next[0] <= ib:
                    tb = qk0_next[0]
                    emit_qk_super(0, tb * P, P)
                    emit_qk_super(JC, tb * P, P)
                    qk0_next[0] += 1

            # prologue: first supers + warmup strips for items 0..LEAD-1
            ensure_bands(0)
            emit_v(0, 0)
            from collections import deque

            def lead_at(idx):
                # shallow lead while head 0 is x-DMA-paced, deep afterwards
                if idx < 10:
                    return 8
                return min(LEAD, 8 + 2 * (idx - 9))

            strip_q = deque()
            next_emit = 0

            def emit_due(idx):
                nonlocal next_emit
                due = []
                while next_emit <= min(idx + lead_at(idx), len(items) - 1):
                    nh, nib = items[next_emit]
                    if nh < 2:
                        ensure_bands(nib)
                    due.extend(
                        (nh, nib, off, n) for (off, n) in strip_pieces(nib))
                    next_emit += 1
                return due

            for k in range(8):
                h, ib = items[k]
                if h < 2:
                    ensure_bands(ib)
                strip_q.append(
                    [(emit_strip_piece(h, ib, off, n), off, n)
                     for (off, n) in strip_pieces(ib)]
                )
            next_emit = 8

            for idx, (h, ib) in enumerate(items):
                pieces = strip_q.popleft()
                sp_next = emit_due(idx)
                po65 = ps_po.tile([P, D + 1], F32, tag="po", name=f"po{h}_{ib}")
                emitted = []
                for j in range(max(len(pieces), len(sp_next))):
                    if j < len(sp_next):
                        nh, nib, off, n = sp_next[j]
                        emitted.append(
                            (nh, nib, (emit_strip_piece(nh, nib, off, n),
                                       off, n)))
                    if j < len(pieces):
                        exs, off, n = pieces[j]
                        emit_pv_piece(h, ib, po65, exs, off, n)
                # regroup emitted pieces by their item, in item order
                cur = None
                for nh, nib, rec in emitted:
                    if cur != (nh, nib):
                        strip_q.append([])
                        cur = (nh, nib)
                    strip_q[-1].append(rec)
                attn_raw = head_tiles(h)[0]
                nc.vector.tensor_copy(attn_raw[:, ib, :], po65[:])
                for fn in inserts.get(idx, []):
                    fn()
                if ib == HB - 1:
                    emit_norm(h, 0)
                elif ib == LC - 1:
                    emit_norm(h, 1)

            # last transpose half can't hide in a later head
            emit_transposes(JC - 1, 1)

            # ---------------- output projection (blocks 6+) ----------------
            for tb in range(6, LC):
                emit_outproj_block(tb, act_drain=True, tail=(tb >= LC - 4))

    nc.compile()
    return nc


_CACHE = {}

# Set by test harnesses to capture a profile; harmless defaults for grading.
TRACE = False
LAST_RESULT = None


def get_program(T=2048):
    if T not in _CACHE:
        _CACHE[T] = build_program(T)
    return _CACHE[T]


def make_in_map(x_b, w_qkv, w_out, hg, T=2048):
    """Host-side shard prep for one core: batch slice x_b [T, C], head group hg."""
    xT = np.ascontiguousarray(x_b.T).astype(NPBF16).reshape(CC, P, T)
    x_t = np.ascontiguousarray(xT.transpose(1, 0, 2))  # [P, CC, T]
    wq = w_qkv[hg * J : (hg + 1) * J]                  # [512, C]
    wk = w_qkv[C + hg * J : C + (hg + 1) * J]
    wv = w_qkv[2 * C + hg * J : 2 * C + (hg + 1) * J]
    # w_sb column order: [q0 k0 | v0 v1 v2 v3 | q1 k1 | q2 k2 | q3 k3]
    W = np.concatenate(
        [wq[0:128], wk[0:128], wv]
        + [np.concatenate([wq[p * 128 : (p + 1) * 128],
                           wk[p * 128 : (p + 1) * 128]])
           for p in range(1, JC)],
        axis=0,
    )  # [3J, C]
    # w_d[p, cc, col] = W[col, cc*128+p]
    wj = np.ascontiguousarray(
        W.T.astype(NPBF16).reshape(CC, P, 3 * J).transpose(1, 0, 2)
    )
    Wo = w_out[:, hg * J : (hg + 1) * J]  # [C, J]
    woutT = np.ascontiguousarray(
        Wo.T.astype(NPBF16).reshape(JC, P, C).transpose(1, 0, 2)
    )
    tri = np.triu(np.ones((P, P), np.float32)).astype(NPBF16)
    eye = np.eye(P, dtype=np.float32).astype(NPBF16)
    return {"wj": wj, "xt": x_t, "woutT": woutT, "trimask": tri, "eye": eye}


def kernel(x, w_qkv, w_out, b_out):
    x = np.asarray(x, dtype=np.float32)
    w_qkv = np.asarray(w_qkv, dtype=np.float32)
    w_out = np.asarray(w_out, dtype=np.float32)
    b_out = np.asarray(b_out, dtype=np.float32)
    B, T, Cx = x.shape
    assert Cx == C

    nc = get_program(T)
    in_maps = [
        make_in_map(x[core // 2], w_qkv, w_out, core % 2, T) for core in range(8)
    ]
    res = run_bass_kernel_spmd(nc, in_maps, core_ids=list(range(8)), trace=TRACE)
    global LAST_RESULT
    LAST_RESULT = res
    outs = [r["y"].astype(np.float32).reshape(T, C) for r in res.results]
    y = np.stack([outs[2 * b] + outs[2 * b + 1] for b in range(B)])
    return (y + b_out[None, None, :]).astype(np.float32)


# revision 79
# speedup vs baseline: 1.1489x; 1.0027x over previous
"""Causal self-attention Bass/Tile kernel for Trainium2, 8 NeuronCores SPMD.

Problem: B=4, T=2048, C=1024, H=16 heads, D=64, f32 in/out.
    qkv = x @ w_qkv.T; per-head causal softmax(q k^T / sqrt(D)) @ v;
    out = attn @ w_out.T + b_out.

Sharding (hybrid batch x tensor-parallel): core c handles batch b = c//2 and
head group hg = c%2 (8 of 16 heads). Each core computes a full [T, C] partial
of the output projection restricted to its heads; the host sums the two
partials per batch (bf16 partials, f32 sum) and adds the bias.

Per-core device algorithm (all matmuls bf16 x bf16 -> f32 PSUM):
  - Inputs arrive as separate w (priority-ordered columns) and x (fine t-band)
    streams on ONE DMA queue so the first score strip issues ~4us in: w[q0],
    x[t<128], w[k0], w[v0] land first, the rest trickles in while head 0
    runs.  Each dma_start costs ~0.6us of descriptor generation, so the
    count is kept low and strictly priority-ordered.
  - qkT is produced in [j, t] layout; chunk-0 supers are emitted on demand
    128 columns at a time right before the strip that needs them (avoids
    head-of-line blocking of the in-order PE queue on the x DMA); later
    chunks go as 512-wide supers.  v is produced in [t, j] layout per
    (block, chunk) with an appended ones column (DVE-broadcast once) for
    the softmax denominators.
  - Attention is a single flat pipeline over items (h, ib) with a ramped
    strip prefetch: score strips are computed ahead of their PV consumption
    (k-stationary, <=8-l-block pieces in 2-bank PSUM tiles), exp'd on
    ScalarE (scale=1/8 folded, no max subtraction), causal-masked on the
    diagonal block (GpSimd), while the PV of item k accumulates
    po65[i, 0:65] += ex_strip[lb]^T @ v_aug[lb] into a 1-bank accumulator
    whose 65th column collects the denominators.  The lead ramps 8 -> 10
    items (22 exs buffers): deep enough to decouple PV from exp-queue
    jitter, shallow enough at the start that head-0 strips don't
    head-of-line-block the in-order PE queue on unarrived x bands; the
    binding constraint is the 3-slot strip-psum rotation (freed at exp's
    deferred ack).
  - po65 drains raw to SBUF per ib (DVE); normalization is batched per
    half-head on GpSimd (dens gather + broadcast multiply; reciprocal on
    DVE) so the DVE queue never delays po65/qkT drains.
  - attn_n[i, (pair-packed j)] is transposed back to attnT[j, i] with PE
    transpose instructions (bf16 PSUM staging), a head PAIR per [128, 128]
    transpose.
  - QKV supers, v chunks, and transposes are placed by a deadline-balanced
    greedy scheduler (earliest = DMA arrival estimate, latest = first use)
    so per-item PE load stays above the ScalarE exp pace in every head.
  - Output projection from attnT with K=128 chunks; the first 6 token
    blocks thread into head 7's tail; PSUM->SBUF drains are split into two
    512 halves (DVE + ACT) because the psum slot is held until the copy's
    deferred ack; y is stored/DMA'd in BF16 (host sums partials in f32),
    halving the output-DMA tail.

PSUM budget (8 banks): score strips 3x2 (shared with out-proj psum), po65/
filler/transpose rotation 2x1.
"""

import sys

if "/opt/trn_rl_repo" not in sys.path:
    sys.path.insert(0, "/opt/trn_rl_repo")

import numpy as np
import ml_dtypes

import concourse.tile as tile
import concourse.mybir as mybir
from concourse import bacc
from concourse.bass_utils import run_bass_kernel_spmd

BF16 = mybir.dt.bfloat16
F32 = mybir.dt.float32
NPBF16 = ml_dtypes.bfloat16
EXPF = mybir.ActivationFunctionType.Exp

P = 128
C = 1024
CC = C // P      # 8 contraction chunks
NH = 8           # heads per core
D = 64
J = NH * D       # 512 (local q/k/v width)
JC = J // P      # 4 j-chunks

LEAD = 10        # strip prefetch depth (items)


def build_program(T=2048):
    LC = T // P          # l/t 128-blocks
    NS = T // 512        # 512-wide t-supers
    HB = LC // 2         # half-head block count (8)
    SCALE = 0.125        # 1/sqrt(D)

    nc = bacc.Bacc("TRN2", target_bir_lowering=False, debug=False, num_devices=8)

    w_d = nc.dram_tensor("wj", [P, CC, 3 * J], BF16, kind="ExternalInput")
    x_d = nc.dram_tensor("xt", [P, CC, T], BF16, kind="ExternalInput")
    woutT_d = nc.dram_tensor("woutT", [P, JC, C], BF16, kind="ExternalInput")
    mask_d = nc.dram_tensor("trimask", [P, P], BF16, kind="ExternalInput")
    eye_d = nc.dram_tensor("eye", [P, P], BF16, kind="ExternalInput")
    y_d = nc.dram_tensor("y", [LC, P, C], BF16, kind="ExternalOutput")

    # x DMA bands (t ranges); finer bands unlock strip/v work sooner
    XBANDS = [(0, 128), (128, 256), (256, 512), (512, 768), (768, 1024),
              (1024, 1280), (1280, 1536), (1536, 1792), (1792, 2048)]

    with tile.TileContext(nc) as tc:
        with (
            tc.tile_pool(name="persist", bufs=1) as persist,
            tc.tile_pool(name="dn", bufs=2) as dn_pool,
            tc.tile_pool(name="expp", bufs=22) as exp_pool,
            tc.tile_pool(name="outp", bufs=4) as out_pool,
            tc.tile_pool(name="ps_sc", bufs=3, space="PSUM") as ps_sc,
            tc.tile_pool(name="ps_po", bufs=2, space="PSUM") as ps_po,
        ):
            w_sb = persist.tile([P, CC, 3 * J], BF16)
            x_sb = persist.tile([P, CC, T], BF16)
            woutT = persist.tile([P, JC, C], BF16)
            trimask = persist.tile([P, P], BF16)
            eye = persist.tile([P, P], BF16)
            qkT = persist.tile([P, 2 * JC, T], BF16)
            v_aug = persist.tile([P, LC, NH, D + 1], BF16)
            # normalized attention in [i, j] layout; head pair p packs its two
            # heads into one 128-wide slab so a single PE transpose covers both
            attn_n = persist.tile([P, LC, JC, P], BF16)
            attnT = persist.tile([P, JC, T], BF16)

            ones1 = persist.tile([P, 1], BF16)
            nc.vector.memset(ones1[:], 1.0)
            nc.vector.tensor_copy(
                v_aug[:, :, :, D], ones1[:, 0].broadcast_to((P, LC, NH))
            )

            # ---- input DMA stream, ordered for earliest first strip ----
            # w_sb column order: [q0 k0 | v0..v3 | q1 k1 | q2 k2 | q3 k3] so
            # priority ranges are contiguous.  One queue = strict DMA order;
            # each dma_start costs ~0.6us of descriptor generation, so keep
            # the count low and front-load only what unblocks compute.
            def wslice(c0, c1):
                nc.sync.dma_start(w_sb[:, :, c0:c1], w_d[:, :, c0:c1])

            def xslice(t0, t1):
                nc.sync.dma_start(x_sb[:, :, t0:t1], x_d[:, :, t0:t1])

            wslice(0, 128)            # q0
            xslice(0, 128)
            wslice(128, 256)          # k0
            nc.scalar.dma_start(trimask[:], mask_d[:])
            wslice(256, 384)          # v chunk 0 (heads 0-1)
            xslice(128, 256)
            xslice(256, 512)
            xslice(512, 768)
            xslice(768, 1024)
            wslice(768, 1024)         # q1, k1
            xslice(1024, 1280)
            xslice(1280, 1536)
            wslice(384, 768)          # v chunks 1-3
            xslice(1536, 1792)
            xslice(1792, 2048)
            wslice(1024, 1536)        # q2 k2 q3 k3
            nc.scalar.dma_start(eye[:], eye_d[:])
            nc.scalar.dma_start(woutT[:], woutT_d[:])

            QCOL = [0, 768, 1024, 1280]
            KCOL = [128, 896, 1152, 1408]
            VCOL = 256

            def wcol(cc, col, n):
                return w_sb[:, cc, col : col + n]

            # ---------------- QKV projection pieces ----------------
            def emit_qk_super(jc, t0, n):
                """qk chunk jc (0-3 q, 4-7 k), t range [t0, t0+n)."""
                col = QCOL[jc] if jc < JC else KCOL[jc - JC]
                pq = ps_po.tile([P, 512], F32, tag="po", name=f"qk{jc}_{t0}")
                for cc in range(CC):
                    nc.tensor.matmul(
                        pq[:, 0:n],
                        wcol(cc, col, P),
                        x_sb[:, cc, t0 : t0 + n],
                        start=(cc == 0),
                        stop=(cc == CC - 1),
                    )
                nc.vector.tensor_copy(qkT[:, jc, t0 : t0 + n], pq[:, 0:n])

            def emit_v(lc, ch, nch=1):
                """v chunks [ch, ch+nch) for one 128-token block lc."""
                pq = ps_po.tile([P, 512], F32, tag="po", name=f"v{lc}_{ch}")
                n = nch * P
                for cc in range(CC):
                    nc.tensor.matmul(
                        pq[:, 0:n],
                        x_sb[:, cc, lc * P : (lc + 1) * P],
                        wcol(cc, VCOL + ch * P, n),
                        start=(cc == 0),
                        stop=(cc == CC - 1),
                    )
                nc.vector.tensor_copy(
                    v_aug[:, lc, 2 * ch : 2 * ch + 2 * nch, 0:D],
                    pq[:, 0:n].rearrange("p (h d) -> p h d", d=D),
                )

            def emit_transposes(pair, g):
                """attn_n[i, pair] -> attnT[j, i] for 8 i-blocks of one pair."""
                tp = ps_po.tile([P, HB, P], BF16, tag="po", name=f"tp{pair}_{g}")
                for i in range(HB):
                    ib = g * HB + i
                    nc.tensor.transpose(
                        tp[:, i, :], attn_n[:, ib, pair, :], eye[:]
                    )
                nc.vector.tensor_copy(
                    attnT[:, pair, g * HB * P : (g + 1) * HB * P],
                    tp[:].rearrange("p a b -> p (a b)"),
                )

            # ---------------- output projection block ----------------
            def emit_outproj_block(tb, act_drain=False, tail=False,
                                   po_psum=False):
                if po_psum:
                    # threaded blocks: 1-bank po-pool groups per oc half so
                    # the strip rotation keeps all three sc slots
                    ot = out_pool.tile([P, C], BF16, tag="ot", name=f"ot{tb}")
                    for oc in range(2):
                        po1 = ps_po.tile([P, 512], F32, tag="po",
                                         name=f"o_ps{tb}_{oc}")
                        for jc in range(JC):
                            nc.tensor.matmul(
                                po1[:],
                                attnT[:, jc, tb * P : (tb + 1) * P],
                                woutT[:, jc, oc * 512 : (oc + 1) * 512],
                                start=(jc == 0),
                                stop=(jc == JC - 1),
                            )
                        nc.vector.tensor_copy(
                            ot[:, oc * 512 : (oc + 1) * 512], po1[:])
                    nc.sync.dma_start(y_d[tb], ot[:])
                    return
                po_ = ps_sc.tile([P, 2, 512], F32, tag="sc", name=f"o_ps{tb}")
                for oc in range(2):
                    for jc in range(JC):
                        nc.tensor.matmul(
                            po_[:, oc, :],
                            attnT[:, jc, tb * P : (tb + 1) * P],
                            woutT[:, jc, oc * 512 : (oc + 1) * 512],
                            start=(jc == 0),
                            stop=(jc == JC - 1),
                        )
                ot = out_pool.tile([P, C], BF16, tag="ot", name=f"ot{tb}")
                # drain in two 512 halves: the psum slot is held until the
                # copy's deferred ack, so one 1024-wide copy would serialize
                # the 3-slot rotation below the PE pace.  act_drain=False
                # keeps ACT free for exp work (threaded blocks).
                nc.vector.tensor_copy(ot[:, 0:512], po_[:, 0, :])
                if act_drain:
                    nc.scalar.copy(ot[:, 512:1024], po_[:, 1, :])
                else:
                    nc.vector.tensor_copy(ot[:, 512:1024], po_[:, 1, :])
                if not tail:
                    nc.sync.dma_start(y_d[tb], ot[:])
                else:
                    # tail: also split the DMA across two queues so the drain
                    # after the last matmul is short
                    nc.sync.dma_start(y_d[tb, :, 0:512], ot[:, 0:512])
                    nc.scalar.dma_start(y_d[tb, :, 512:1024], ot[:, 512:1024])

            # ---------------- insert plan (global item index) ----------------
            # an entry at key k is emitted after item k's PV drain
            inserts = {}
            NITEM = NH * LC

            def add_ins(idx, fn):
                inserts.setdefault(idx, []).append(fn)

            items = [(h, ib) for h in range(NH) for ib in range(LC)]

            # per-item base PE load (cycles): strip of item idx+LEAD + PV
            load = [0.0] * NITEM
            for idx in range(NITEM):
                ib = items[idx][1]
                load[idx] = (ib + 1) * 65 + 120  # PV + drain slack
                if idx + LEAD < NITEM:
                    load[idx] += (items[idx + LEAD][1] + 1) * 128  # strip

            # forced inserts: v chunk 0 block lc before PV(0, lc); placed a
            # few items early so it isn't stuck behind later-x strip supers
            for lc in range(1, LC):
                add_ins(max(lc - 3, 0), (lambda lc=lc: emit_v(lc, 0)))
                load[max(lc - 3, 0)] += CC * P + 120
            # q0/k0 supers are emitted on demand (ensure_bands); account the
            # load at the item whose strip emission pulls them in
            for ib in range(LC):
                load[max(ib - LEAD, 0)] += 2 * CC * P

            # pinned late-stage work: tp(3,0) right after norm(7,0), then the
            # first 6 out-proj token blocks thread into head 7's tail
            add_ins(121, (lambda: emit_transposes(JC - 1, 0)))
            load[121] += HB * P + 120
            for tb in range(4):
                add_ins(124 + tb,
                        (lambda tb=tb: emit_outproj_block(tb, po_psum=True)))
                load[124 + tb] += 2 * JC * 512

            # deadline-balanced inserts: (earliest, latest, cycles, fn)
            # earliest indices approximate when the needed x/w DMA has landed
            xband_idx = {0: 0, 1: 1, 2: 3, 3: 5, 4: 7, 5: 8, 6: 9,
                         7: 10, 8: 11}

            def lc_band(lc):
                for bi, (t0, t1) in enumerate(XBANDS):
                    if lc * P < t1:
                        return bi
                return len(XBANDS) - 1

            cand = []
            for ch in (1, 2, 3):
                for lc in range(LC):
                    # v(lc, ch) used by PV(2ch, lc) at item 16*2ch + lc
                    earliest = max(xband_idx[lc_band(lc)], 7)
                    cand.append((earliest, 32 * ch + lc - 1, CC * P + 120,
                                 (lambda lc=lc, ch=ch: emit_v(lc, ch))))
            for pair in range(1, JC):
                w_idx = 6 if pair == 1 else 13
                for tsb in range(NS):
                    earliest = max(xband_idx[tsb + 1], w_idx)
                    latest = 32 * pair + 4 * tsb - LEAD - 1
                    cand.append((earliest, latest, CC * 512,
                                 (lambda p=pair, t=tsb:
                                  emit_qk_super(JC + p, t * 512, 512))))
                    cand.append((earliest, latest, CC * 512,
                                 (lambda p=pair, t=tsb:
                                  emit_qk_super(p, t * 512, 512))))
            for pair in range(JC):
                e0 = 16 * (2 * pair + 1) + 8 + 1   # after norm(2p+1, 0)
                e1 = 16 * (2 * pair + 2)           # after norm(2p+1, 1)
                if pair < JC - 1:
                    cand.append((e0, 118, HB * P + 120,
                                 (lambda p=pair: emit_transposes(p, 0))))
                    cand.append((e1, NITEM - 1, HB * P + 120,
                                 (lambda p=pair: emit_transposes(p, 1))))

            cand.sort(key=lambda e: e[1])
            for earliest, latest, cost, fn in cand:
                lo = max(0, min(earliest, NITEM - 1))
                hi = max(lo, min(latest, NITEM - 1))
                k = min(range(lo, hi + 1), key=lambda i: load[i])
                add_ins(k, fn)
                load[k] += cost

            # ---------------- attention (flat item pipeline) ----------------
            def strip_pieces(ib):
                n = ib + 1
                return [(0, n)] if n <= HB else [(0, HB), (HB, n - HB)]

            def qk_head(h):
                bp = (h % 2) * 64
                chq = h // 2
                return (qkT[bp : bp + 64, chq, :], qkT[bp : bp + 64, JC + chq, :])

            def emit_strip_piece(h, ib, off, n):
                """scores + exp (+ diag mask) for l-blocks [off, off+n) of
                query block ib of head h; returns the bf16 exp strip."""
                qTh, kTh = qk_head(h)
                st = ps_sc.tile([P, HB, P], F32, tag="sc",
                                name=f"st{h}_{ib}_{off}")
                for i in range(n):
                    lb = off + i
                    nc.tensor.matmul(
                        st[:, i, :],
                        kTh[:, lb * P : (lb + 1) * P],
                        qTh[:, ib * P : (ib + 1) * P],
                        start=True, stop=True,
                    )
                exs = exp_pool.tile([P, HB, P], BF16, tag="ex",
                                    name=f"ex{h}_{ib}_{off}")
                nc.scalar.activation(
                    exs[:, 0:n, :].rearrange("p a b -> p (a b)"),
                    st[:, 0:n, :].rearrange("p a b -> p (a b)"),
                    EXPF, scale=SCALE,
                )
                if off <= ib < off + n:
                    nc.gpsimd.tensor_mul(
                        exs[:, ib - off, :], exs[:, ib - off, :], trimask[:]
                    )
                return exs

            def emit_pv_piece(h, ib, po65, exs, off, n):
                for i in range(n):
                    lb = off + i
                    nc.tensor.matmul(
                        po65[:], exs[:, i, :], v_aug[:, lb, h, :],
                        start=(lb == 0), stop=(lb == ib),
                    )

            heads = {}

            def head_tiles(h):
                if h not in heads:
                    attn_raw = dn_pool.tile([P, LC, D + 1], F32, tag="ar",
                                            name=f"ar{h}")
                    dens = dn_pool.tile([P, LC], F32, tag="dn", name=f"dn{h}")
                    recips = dn_pool.tile([P, LC], F32, tag="rc", name=f"rc{h}")
                    rscr = dn_pool.tile([P, LC], F32, tag="rs", name=f"rs{h}")
                    heads[h] = (attn_raw, dens, recips, rscr)
                return heads[h]

            def emit_norm(h, half):
                attn_raw, dens, recips, rscr = head_tiles(h)
                bp = (h % 2) * 64
                chq = h // 2
                s = half * HB
                e = s + HB
                nc.gpsimd.tensor_copy(dens[:, s:e], attn_raw[:, s:e, D])
                nc.vector.reciprocal_approx_accurate(
                    recips[:, s:e], dens[:, s:e], rscr[:, s:e]
                )
                nc.gpsimd.tensor_mul(
                    attn_n[:, s:e, chq, bp : bp + 64],
                    attn_raw[:, s:e, 0:D],
                    recips[:, s:e].broadcast_to((P, HB, D)),
                )

            # q0/k0 supers emitted on demand, one 128-col block at a time,
            # right before the strip needing them: fine granularity avoids
            # head-of-line blocking of the in-order PE queue on the x DMA
            qk0_next = [0]

            def ensure_bands(ib):
                while qk0_next[0] <= ib:
                    tb = qk0_next[0]
                    emit_qk_super(0, tb * P, P)
                    emit_qk_super(JC, tb * P, P)
                    qk0_next[0] += 1

            # prologue: first supers + warmup strips for items 0..LEAD-1
            ensure_bands(0)
            emit_v(0, 0)
            from collections import deque

            def lead_at(idx):
                # shallow lead while head 0 is x-DMA-paced, deep afterwards
                if idx < 10:
                    return 8
                return min(LEAD, 8 + 2 * (idx - 9))

            strip_q = deque()
            next_emit = 0

            def emit_due(idx):
                nonlocal next_emit
                due = []
                while next_emit <= min(idx + lead_at(idx), len(items) - 1):
                    nh, nib = items[next_emit]
                    if nh < 2:
                        ensure_bands(nib)
                    due.extend(
                        (nh, nib, off, n) for (off, n) in strip_pieces(nib))
                    next_emit += 1
                return due

            for k in range(8):
                h, ib = items[k]
                if h < 2:
                    ensure_bands(ib)
                strip_q.append(
                    [(emit_strip_piece(h, ib, off, n), off, n)
                     for (off, n) in strip_pieces(ib)]
                )
            next_emit = 8

            for idx, (h, ib) in enumerate(items):
                pieces = strip_q.popleft()
                sp_next = emit_due(idx)
                po65 = ps_po.tile([P, D + 1], F32, tag="po", name=f"po{h}_{ib}")
                emitted = []
                for j in range(max(len(pieces), len(sp_next))):
                    if j < len(sp_next):
                        nh, nib, off, n = sp_next[j]
                        emitted.append(
                            (nh, nib, (emit_strip_piece(nh, nib, off, n),
                                       off, n)))
                    if j < len(pieces):
                        exs, off, n = pieces[j]
                        emit_pv_piece(h, ib, po65, exs, off, n)
                # regroup emitted pieces by their item, in item order
                cur = None
                for nh, nib, rec in emitted:
                    if cur != (nh, nib):
                        strip_q.append([])
                        cur = (nh, nib)
                    strip_q[-1].append(rec)
                attn_raw = head_tiles(h)[0]
                nc.vector.tensor_copy(attn_raw[:, ib, :], po65[:])
                for fn in inserts.get(idx, []):
                    fn()
                if ib == HB - 1:
                    emit_norm(h, 0)
                elif ib == LC - 1:
                    emit_norm(h, 1)

            # last transpose half can't hide in a later head
            emit_transposes(JC - 1, 1)

            # ---------------- output projection (blocks 6+) ----------------
            for tb in range(4, LC):
                emit_outproj_block(tb, act_drain=True, tail=(tb >= LC - 4))

    nc.compile()
    return nc


_CACHE = {}

# Set by test harnesses to capture a profile; harmless defaults for grading.
TRACE = False
LAST_RESULT = None


def get_program(T=2048):
    if T not in _CACHE:
        _CACHE[T] = build_program(T)
    return _CACHE[T]


def make_in_map(x_b, w_qkv, w_out, hg, T=2048):
    """Host-side shard prep for one core: batch slice x_b [T, C], head group hg."""
    xT = np.ascontiguousarray(x_b.T).astype(NPBF16).reshape(CC, P, T)
    x_t = np.ascontiguousarray(xT.transpose(1, 0, 2))  # [P, CC, T]
    wq = w_qkv[hg * J : (hg + 1) * J]                  # [512, C]
    wk = w_qkv[C + hg * J : C + (hg + 1) * J]
    wv = w_qkv[2 * C + hg * J : 2 * C + (hg + 1) * J]
    # w_sb column order: [q0 k0 | v0 v1 v2 v3 | q1 k1 | q2 k2 | q3 k3]
    W = np.concatenate(
        [wq[0:128], wk[0:128], wv]
        + [np.concatenate([wq[p * 128 : (p + 1) * 128],
                           wk[p * 128 : (p + 1) * 128]])
           for p in range(1, JC)],
        axis=0,
    )  # [3J, C]
    # w_d[p, cc, col] = W[col, cc*128+p]
    wj = np.ascontiguousarray(
        W.T.astype(NPBF16).reshape(CC, P, 3 * J).transpose(1, 0, 2)
    )
    Wo = w_out[:, hg * J : (hg + 1) * J]  # [C, J]
    woutT = np.ascontiguousarray(
        Wo.T.astype(NPBF16).reshape(JC, P, C).transpose(1, 0, 2)
    )
    tri = np.triu(np.ones((P, P), np.float32)).astype(NPBF16)
    eye = np.eye(P, dtype=np.float32).astype(NPBF16)
    return {"wj": wj, "xt": x_t, "woutT": woutT, "trimask": tri, "eye": eye}


def kernel(x, w_qkv, w_out, b_out):
    x = np.asarray(x, dtype=np.float32)
    w_qkv = np.asarray(w_qkv, dtype=np.float32)
    w_out = np.asarray(w_out, dtype=np.float32)
    b_out = np.asarray(b_out, dtype=np.float32)
    B, T, Cx = x.shape
    assert Cx == C

    nc = get_program(T)
    in_maps = [
        make_in_map(x[core // 2], w_qkv, w_out, core % 2, T) for core in range(8)
    ]
    res = run_bass_kernel_spmd(nc, in_maps, core_ids=list(range(8)), trace=TRACE)
    global LAST_RESULT
    LAST_RESULT = res
    outs = [r["y"].astype(np.float32).reshape(T, C) for r in res.results]
    y = np.stack([outs[2 * b] + outs[2 * b + 1] for b in range(B)])
    return (y + b_out[None, None, :]).astype(np.float32)
